# revision 3
# baseline (speedup 1.0000x reference)
"""Trainium2 Bass kernel for nn_MultiHeadQuantileNBEATS.

Reference computation (per batch row b):
  feats = x @ W_bb + b_bb                                   [D]
  h1[q] = relu(feats @ W1[q] + b1[q])                       [QF, H1]
  h2[q] = relu(h1[q] @ W2[q] + b2[q])                       [QF, H2]
  o3[q] = h2[q] @ W3[q] + b3[q]                             [QF, HOR]
  sq    = sort(o3 over q)  (per (b, hor))                   [HOR, QF]
  out[b, h, t] = sort_t(interp(sq[b, h, :], q[b, t]))       [HOR, QT]

Device algorithm notes:
  * Pure data parallel over 8 cores (batch sharded, weights replicated).
  * The backbone is folded into the first head layer on the HOST:
      W1c[q] = W_bb @ W1[q],  b1c[q] = b_bb @ W1[q] + b1[q]
    so the device computes h1 = relu(x @ W1c + b1c) directly.
  * x is pre-transposed on the host ([T, B_core]) so its SBUF layout is
    feature-major with no on-chip transposes.
  * All head math is "feature major": activations stored [feature, batch]
    so weights are used as-stored (lhsT = W) and chained matmuls need no
    transposes.
  * The final sort over the QT axis is eliminated: the quantile interpolant
    is monotone non-decreasing in the query level, so sorting q per row
    first yields an output that is already sorted.
  * Interpolation r[b,h,t] = sum_i a_i(q[b,t]) * sq_i[b,h] is one K=112 PE
    matmul per group of 16 samples: lhsT = transposed sorted head outputs
    [(s,i), h], rhs = block-diagonal coefficient matrix [(s,i), (s,t)].
  * Two-supertile software pipeline: PE order is heads(0), heads(1),
    interp(0), interp(1); the DVE runs the 7-way sort of supertile st
    while the PE computes heads(st+1), so the PE never idles and the HAM
    clock gate stays warm.  The interp coefficient build (ain, A) runs on
    the otherwise-idle GPSIMD engine so it never contends with the sorts.
  * Per-core output is written feature-major [HOR, B_core, QT] (contiguous
    DMA); the host transposes to [B, HOR, QT] when gathering.
"""

import dataclasses
from contextlib import ExitStack

import numpy as np

import concourse.bass as bass
import concourse.mybir as mybir
import concourse.tile as tile
from concourse import bass_utils
from concourse.bass import ts
from concourse.masks import make_identity

F32 = mybir.dt.float32

B, T, D = 8192, 512, 512
H1, H2, HOR = 256, 128, 96
QF, QT = 7, 32
NCORES = 8
BC = B // NCORES  # batch per core
SUB = 512         # samples per super-tile
QUANTILE_LEVELS = np.array(
    [0.025, 0.1, 0.25, 0.5, 0.75, 0.9, 0.975], dtype=np.float32
)

# ---------------------------------------------------------------------------
# sorting networks
# ---------------------------------------------------------------------------

# optimal 16-CE sorting network for 7 elements (ascending), disjoint layers
SORT7_LAYERS = [
    [(1, 2), (3, 4), (5, 6)],
    [(0, 2), (3, 5), (4, 6)],
    [(0, 1), (4, 5), (2, 6)],
    [(0, 4), (1, 5)],
    [(0, 3), (2, 5)],
    [(1, 3), (2, 4)],
    [(2, 3)],
]


def batcher_layers(n):
    """Batcher odd-even mergesort compare-exchange layers (uniform distance)."""
    layers = []
    p = 1
    while p < n:
        k = p
        while k >= 1:
            pairs = []
            j = k % p
            while j + k < n:
                for i in range(min(k, n - j - k)):
                    lo, hi = i + j, i + j + k
                    if lo // (2 * p) == hi // (2 * p):
                        pairs.append((lo, hi))
                j += 2 * k
            if pairs:
                layers.append(pairs)
            k //= 2
        p *= 2
    return layers


def coalesce2(indices):
    """Coalesce sorted ints into 2-level lattices (start, ostep, ocount, icount)."""
    idx = list(indices)
    runs = []
    i = 0
    while i < len(idx):
        cnt = 1
        while i + cnt < len(idx) and idx[i + cnt] - idx[i + cnt - 1] == 1:
            cnt += 1
        runs.append((idx[i], cnt))
        i += cnt
    groups = []
    i = 0
    while i < len(runs):
        start, cnt = runs[i]
        j = i + 1
        if j < len(runs) and runs[j][1] == cnt:
            ostep = runs[j][0] - start
            k = j
            while (
                k < len(runs)
                and runs[k][1] == cnt
                and runs[k][0] - runs[k - 1][0] == ostep
            ):
                k += 1
            groups.append((start, ostep, k - i, cnt))
            i = k
        else:
            groups.append((start, 1, 1, cnt))
            i += 1
    return groups


def _view(ap, free_dims, extra_offset):
    """Rebuild an AP keeping its partition dim, with custom free-dim lattice."""
    dims = [tuple(ap.ap[0])] + [tuple(d) for d in free_dims]
    return dataclasses.replace(ap, ap=tuple(dims), offset=ap.offset + extra_offset)


# ---------------------------------------------------------------------------
# host-side constants
# ---------------------------------------------------------------------------

def _host_constants(b1c, b2, b3):
    # bias_all [128, 32]: packed per-partition bias columns
    bias = np.zeros((128, 32), dtype=np.float32)
    for qh in range(QF):
        for mc in range(H1 // 128):
            bias[:, 2 * qh + mc] = b1c[qh, 128 * mc : 128 * (mc + 1)]
        bias[:, 14 + qh] = b2[qh]
        bias[:96, 21 + qh] = b3[qh]
    # M112 [112, 512]: block-diagonal 0/1 mask over (sample, coeff) x (sample, t)
    m112 = np.zeros((112, 512), dtype=np.float32)
    for s in range(16):
        m112[7 * s : 7 * s + 7, 32 * s : 32 * s + 32] = 1.0
    return bias, m112


# ---------------------------------------------------------------------------
# device kernel
# ---------------------------------------------------------------------------

def _emit(ctx: ExitStack, tc: tile.TileContext, ins, outs, bc=BC):
    nc = tc.nc
    xT_d, q_d, w1_d, w2_d, w3_d, bias_d, m112_d = ins
    (r_d,) = outs
    n_sub = bc // SUB
    n_chunk = bc // 128
    ngrp = SUB // 16
    ql = QUANTILE_LEVELS

    cpool = ctx.enter_context(tc.tile_pool(name="cpool", bufs=1))
    wpool = ctx.enter_context(tc.tile_pool(name="wpool", bufs=1))
    qpool = ctx.enter_context(tc.tile_pool(name="qpool", bufs=1))
    xpool = ctx.enter_context(tc.tile_pool(name="xpool", bufs=1))
    h1pool = ctx.enter_context(tc.tile_pool(name="h1pool", bufs=1))
    h2pool = ctx.enter_context(tc.tile_pool(name="h2pool", bufs=1))
    o3pool = ctx.enter_context(tc.tile_pool(name="o3pool", bufs=9))
    sqgpool = ctx.enter_context(tc.tile_pool(name="sqgpool", bufs=2))
    sqTpool = ctx.enter_context(tc.tile_pool(name="sqTpool", bufs=4))
    apool = ctx.enter_context(tc.tile_pool(name="apool", bufs=4))
    rpool = ctx.enter_context(tc.tile_pool(name="rpool", bufs=4))
    tpsum = ctx.enter_context(tc.tile_pool(name="tpsum", bufs=3, space="PSUM"))
    hpsum = ctx.enter_context(tc.tile_pool(name="hpsum", bufs=2, space="PSUM"))
    rpsum = ctx.enter_context(tc.tile_pool(name="rpsum", bufs=2, space="PSUM"))

    # --- constants ---
    ident = cpool.tile([128, 128], F32)
    make_identity(nc, ident[:])
    bias_sb = cpool.tile([128, 32], F32)
    nc.sync.dma_start(bias_sb[:], bias_d)
    m112 = cpool.tile([112, 512], F32)
    nc.sync.dma_start(m112[:], m112_d)

    # PE warm-up: absorb the GPSIMD (identity build) clock into PE's vector
    # clock so later transposes don't accumulate a third sync wait.
    warm_ps = tpsum.tile([128, 128], F32, tag="tps")
    nc.tensor.matmul(warm_ps[:], lhsT=ident[:], rhs=ident[:], start=True, stop=True)

    # --- input / weight DMAs, ordered so the PE can start early ---
    # x^T chunks for supertile 0 first, then W1 head by head, so the first
    # W1 matmul only waits on ~1.2 MiB of DMA.
    xT_sb = [[None] * (T // 128) for _ in range(n_sub)]
    for st in range(n_sub):
        for tci in range(T // 128):
            xt = xpool.tile([128, SUB], F32, name=f"xT{st}_{tci}")
            nc.sync.dma_start(xt[:], xT_d[ts(tci, 128), ts(st, SUB)])
            xT_sb[st][tci] = xt
        if st == 0:
            w1_sb = []
            for qh in range(QF):
                w = wpool.tile([128, (D // 128) * H1], F32, name=f"w1_{qh}")
                nc.sync.dma_start(
                    w[:].rearrange("p (c m) -> p c m", c=D // 128),
                    w1_d[qh].rearrange("(c p) m -> p c m", c=D // 128),
                )
                w1_sb.append(w)

    q_all = qpool.tile([128, n_chunk * QT], F32)
    nc.sync.dma_start(
        q_all[:].rearrange("p (c t) -> p c t", c=n_chunk),
        q_d.rearrange("(c p) t -> p c t", c=n_chunk),
    )

    w2_sb = []
    for qh in range(QF):
        w = wpool.tile([128, (H1 // 128) * H2], F32, name=f"w2_{qh}")
        nc.sync.dma_start(
            w[:].rearrange("p (c m) -> p c m", c=H1 // 128),
            w2_d[qh].rearrange("(c p) m -> p c m", c=H1 // 128),
        )
        w2_sb.append(w)
    w3_sb = []
    for qh in range(QF):
        w = wpool.tile([128, HOR], F32, name=f"w3_{qh}")
        nc.sync.dma_start(w[:], w3_d[qh])
        w3_sb.append(w)

    # =====================================================================
    # q path: per-row sort over QT on the DVE (overlaps the weight DMAs)
    # =====================================================================
    qscratch = qpool.tile([128, n_chunk * QT], F32)
    for layer in batcher_layers(QT):
        dist = layer[0][1] - layer[0][0]
        for (start, ostep, ocount, icount) in coalesce2(
            sorted(a for a, _ in layer)
        ):
            fd = [(QT, n_chunk), (ostep, ocount), (1, icount)]
            lo = _view(q_all[:], fd, start)
            hi = _view(q_all[:], fd, start + dist)
            sc = _view(qscratch[:], [(QT, n_chunk), (icount, ocount), (1, icount)], 0)
            nc.vector.tensor_tensor(sc, lo, hi, op=mybir.AluOpType.min)
            nc.vector.tensor_tensor(hi, lo, hi, op=mybir.AluOpType.max)
            nc.vector.tensor_copy(lo, sc)

    # =====================================================================
    # emission helpers (pipelined: heads(0), heads(1), interp(0), interp(1))
    # =====================================================================

    def emit_qT_ain(st):
        """Transpose sorted q for this supertile (PE) and build the
        interpolation coefficients ain [QT, 7*SUB] on the GPSIMD."""
        qT = qpool.tile([QT, SUB], F32, name=f"qT{st}", tag="qT", bufs=2)
        for c in range(SUB // 128):
            ps = tpsum.tile([QT, 128], F32, tag="tps")
            nc.tensor.matmul(
                ps[:], lhsT=q_all[:, ts(st * (SUB // 128) + c, QT)],
                rhs=ident[:], start=True, stop=True,
            )
            nc.scalar.copy(qT[:, ts(c, 128)], ps[:])

        # a_i = f_i - f_{i+1} (f_0 = 1, f_7 = 0) written into AIN [32, 7*SUB]
        # at stride 7 so that column 7*s + i holds a_i(sample s).
        ain = qpool.tile([QT, 7 * SUB], F32, name=f"ain{st}", tag="ain", bufs=2)

        def ain_view(i):
            return _view(ain[:], [(7, SUB)], i)

        f_prev = None
        for i in range(1, 7):
            lam = float(ql[i - 1])
            inv = float(
                np.float32(1.0)
                / (np.float32(ql[i] - ql[i - 1]) + np.float32(1e-8))
            )
            u = qpool.tile([QT, SUB], F32, name=f"u{st}_{i}", tag="utile", bufs=2)
            nc.gpsimd.tensor_scalar(
                u[:], qT[:], lam, inv,
                mybir.AluOpType.subtract, mybir.AluOpType.mult,
            )
            f = qpool.tile([QT, SUB], F32, name=f"f{st}_{i}", tag=f"fp{i % 2}",
                           bufs=1)
            nc.gpsimd.tensor_scalar(
                f[:], u[:], 1.0, 0.0, mybir.AluOpType.min, mybir.AluOpType.max
            )
            if i == 1:
                nc.gpsimd.tensor_scalar(
                    ain_view(0), f[:], -1.0, 1.0,
                    mybir.AluOpType.mult, mybir.AluOpType.add,
                )
            else:
                nc.gpsimd.tensor_tensor(
                    ain_view(i - 1), f_prev[:], f[:], op=mybir.AluOpType.subtract
                )
            f_prev = f
        nc.gpsimd.tensor_copy(ain_view(6), f_prev[:])
        return ain

    def emit_heads(st):
        """h1 -> h2 -> o3 for one supertile on the PE (fp32)."""
        xT = xT_sb[st]
        h1T = [[None] * (H1 // 128) for _ in range(QF)]
        for qh in range(QF):
            for mc in range(H1 // 128):
                ps = hpsum.tile([128, SUB], F32, tag="hps")
                for dc in range(D // 128):
                    nc.tensor.matmul(
                        ps[:],
                        lhsT=w1_sb[qh][:, ts(dc * 2 + mc, 128)],
                        rhs=xT[dc][:],
                        start=(dc == 0),
                        stop=(dc == 3),
                    )
                h1 = h1pool.tile([128, SUB], F32, name=f"h1_{st}_{qh}_{mc}",
                                 tag=f"h1_{qh}_{mc}", bufs=1)
                nc.scalar.activation(
                    h1[:], ps[:], mybir.ActivationFunctionType.Relu,
                    bias=bias_sb[:, 2 * qh + mc : 2 * qh + mc + 1], scale=1.0,
                )
                h1T[qh][mc] = h1

        h2T = [None] * QF
        for qh in range(QF):
            ps = hpsum.tile([128, SUB], F32, tag="hps")
            for mc in range(H1 // 128):
                nc.tensor.matmul(
                    ps[:], lhsT=w2_sb[qh][:, ts(mc, H2)], rhs=h1T[qh][mc][:],
                    start=(mc == 0), stop=(mc == 1),
                )
            h2 = h2pool.tile([128, SUB], F32, name=f"h2_{st}_{qh}",
                             tag=f"h2_{qh}", bufs=1)
            nc.scalar.activation(
                h2[:], ps[:], mybir.ActivationFunctionType.Relu,
                bias=bias_sb[:, 14 + qh : 15 + qh], scale=1.0,
            )
            h2T[qh] = h2

        o3 = [None] * QF
        for qh in range(QF):
            ps = hpsum.tile([HOR, SUB], F32, tag="hps")
            nc.tensor.matmul(
                ps[:], lhsT=w3_sb[qh][:, :], rhs=h2T[qh][:], start=True, stop=True
            )
            o = o3pool.tile([HOR, SUB], F32, name=f"o3_{st}_{qh}", tag="sortt")
            nc.scalar.activation(
                o[:], ps[:], mybir.ActivationFunctionType.Identity,
                bias=bias_sb[:HOR, 21 + qh : 22 + qh], scale=1.0,
            )
            o3[qh] = o
        return o3

    def emit_sort(st, o3):
        """7-element sort network on the DVE; final values land interleaved
        in SQG [96, 32*112] (free idx = 112*g + 7*s + i)."""
        sqg = sqgpool.tile([HOR, ngrp * 112], F32, name=f"sqg{st}", tag="sqg")
        last_touch = {}
        for li, layer in enumerate(SORT7_LAYERS):
            for (a, b) in layer:
                last_touch[a] = (li, a, b)
                last_touch[b] = (li, a, b)
        cur = {k: o3[k] for k in range(QF)}

        def sqg_slot(j):
            return _view(sqg[:], [(112, ngrp), (7, 16)], j)

        ce_idx = 0
        for li, layer in enumerate(SORT7_LAYERS):
            for (a, b) in layer:
                ia = cur[a][:].rearrange("p (g s) -> p g s", g=ngrp)
                ib = cur[b][:].rearrange("p (g s) -> p g s", g=ngrp)
                a_final = last_touch[a] == (li, a, b)
                b_final = last_touch[b] == (li, a, b)
                if a_final:
                    oa = sqg_slot(a)
                else:
                    ta = o3pool.tile([HOR, SUB], F32, name=f"s{st}_{ce_idx}a",
                                     tag="sortt")
                    oa = ta[:].rearrange("p (g s) -> p g s", g=ngrp)
                if b_final:
                    ob = sqg_slot(b)
                else:
                    tb = o3pool.tile([HOR, SUB], F32, name=f"s{st}_{ce_idx}b",
                                     tag="sortt")
                    ob = tb[:].rearrange("p (g s) -> p g s", g=ngrp)
                nc.vector.tensor_tensor(oa, ia, ib, op=mybir.AluOpType.min)
                nc.vector.tensor_tensor(ob, ia, ib, op=mybir.AluOpType.max)
                if not a_final:
                    cur[a] = ta
                if not b_final:
                    cur[b] = tb
                ce_idx += 1
        return sqg

    def emit_interp(st, sqg, ain):
        """Per 16-sample group: PE transposes, GPSIMD A build, PE interp
        matmul, scalar evacuation, DMA out."""
        for g in range(ngrp):
            gg = st * ngrp + g  # global group index

            ps_t = tpsum.tile([112, 128], F32, tag="tps")
            nc.tensor.matmul(
                ps_t[:, :HOR], lhsT=sqg[:, 112 * g : 112 * (g + 1)],
                rhs=ident[:HOR, :HOR], start=True, stop=True,
            )
            nc.tensor.matmul(
                ps_t[:, HOR:128], lhsT=_view(ain[:], [(1, 112)], 112 * g),
                rhs=ident[:QT, :QT], start=True, stop=True,
            )
            sqa = sqTpool.tile([112, 128], F32, tag="sqa")
            nc.scalar.copy(sqa[:], ps_t[:])

            # A [112, 512] = broadcast(aT) * M112 on the GPSIMD
            A = apool.tile([112, 512], F32, tag="A")
            av = sqa[:, HOR:128].unsqueeze(1).broadcast_to((112, 16, QT))
            mv = m112[:].rearrange("p (s t) -> p s t", s=16)
            Av = A[:].rearrange("p (s t) -> p s t", s=16)
            nc.gpsimd.tensor_tensor(Av, av, mv, op=mybir.AluOpType.mult)

            rps = rpsum.tile([HOR, 512], F32, tag="rps")
            nc.tensor.matmul(
                rps[:], lhsT=sqa[:, :HOR], rhs=A[:], start=True, stop=True
            )

            r_sb = rpool.tile([HOR, 512], F32, tag="rsb")
            nc.scalar.copy(r_sb[:], rps[:])
            nc.sync.dma_start(
                r_d[:, 16 * gg : 16 * (gg + 1), :],
                r_sb[:].rearrange("p (s t) -> p s t", s=16),
            )

    # =====================================================================
    # pipelined emission
    # =====================================================================
    o3_0 = emit_heads(0)
    ain = [None] * n_sub
    for st in range(n_sub):
        ain[st] = emit_qT_ain(st)
    sqg_0 = emit_sort(0, o3_0)
    o3_1 = emit_heads(1)
    sqg_1 = emit_sort(1, o3_1)
    emit_interp(0, sqg_0, ain[0])
    emit_interp(1, sqg_1, ain[1])


# Per-instruction-type sync-wait slot capacity in the walrus ISA descriptors.
_WAIT_CAPACITY = {}  # default: every type gets a single wait slot
_DRAIN_CAPACITY = {
    "EngineType.SP": 1,
    "EngineType.PE": 1,
}


def _split_waits(nc):
    """Some walrus ISA descriptors (LDWEIGHTS, DMA) have too few sync-wait
    slots for the waits Tile emits.  Move surplus waits of overflowing
    instructions onto drains inserted right before them on the same queue."""
    for fn in nc.m.functions:
        for blk in fn.blocks:
            insts = list(blk.instructions)
            out = []
            changed = False
            for ins in insts:
                si = ins.sync_info
                cap = _WAIT_CAPACITY.get(type(ins).__name__, 1)
                if si is not None and si.on_wait and len(si.on_wait) > cap:
                    waits = list(si.on_wait)
                    surplus = waits[:-cap]
                    dcap = _DRAIN_CAPACITY.get(str(ins.engine), 1)
                    di = 0
                    while surplus:
                        chunk, surplus = surplus[:dcap], surplus[dcap:]
                        out.append(
                            mybir.InstDrain(
                                name=f"{ins.name}-wfence{di}",
                                engine=ins.engine,
                                ins=[],
                                outs=[],
                                sync_info=mybir.SyncInfo(
                                    on_wait=chunk, on_update=[]
                                ),
                            )
                        )
                        di += 1
                    si.on_wait = waits[-cap:]
                    changed = True
                out.append(ins)
            if changed:
                blk.instructions = out


def build_module(bc=BC):
    nc = bass.Bass("TRN2", target_bir_lowering=False, debug=False)
    xT_d = nc.dram_tensor("xT", [T, bc], F32, kind="ExternalInput").ap()
    q_d = nc.dram_tensor("q", [bc, QT], F32, kind="ExternalInput").ap()
    w1_d = nc.dram_tensor("W1", [QF, D, H1], F32, kind="ExternalInput").ap()
    w2_d = nc.dram_tensor("W2", [QF, H1, H2], F32, kind="ExternalInput").ap()
    w3_d = nc.dram_tensor("W3", [QF, H2, HOR], F32, kind="ExternalInput").ap()
    bias_d = nc.dram_tensor("bias_all", [128, 32], F32, kind="ExternalInput").ap()
    m112_d = nc.dram_tensor("m112", [112, 512], F32, kind="ExternalInput").ap()
    r_d = nc.dram_tensor("r_out", [HOR, bc, QT], F32, kind="ExternalOutput").ap()

    with tile.TileContext(nc) as tc:
        with ExitStack() as ctx:
            _emit(ctx, tc, (xT_d, q_d, w1_d, w2_d, w3_d, bias_d, m112_d),
                  (r_d,), bc=bc)
    _split_waits(nc)
    return nc


_NC_CACHE = {}
LAST_EXEC_TIME_NS = None


def kernel(**inputs) -> np.ndarray:
    global LAST_EXEC_TIME_NS
    x = np.asarray(inputs["x"], dtype=np.float32)
    q = np.asarray(inputs["q"], dtype=np.float32)
    w_bb = np.asarray(inputs["W_bb"], dtype=np.float64)
    b_bb = np.asarray(inputs["b_bb"], dtype=np.float64)
    w1 = np.asarray(inputs["W1"], dtype=np.float64)
    b1 = np.asarray(inputs["b1"], dtype=np.float64)
    w2 = np.ascontiguousarray(np.asarray(inputs["W2"], dtype=np.float32))
    w3 = np.ascontiguousarray(np.asarray(inputs["W3"], dtype=np.float32))

    # Fold the backbone into the first head layer (in float64 on the host).
    w1c = np.ascontiguousarray((w_bb[None, :, :] @ w1).astype(np.float32))
    b1c = np.ascontiguousarray((b_bb @ w1 + b1).astype(np.float32))

    bias, m112 = _host_constants(
        b1c,
        np.asarray(inputs["b2"], dtype=np.float32),
        np.asarray(inputs["b3"], dtype=np.float32),
    )

    if BC not in _NC_CACHE:
        _NC_CACHE[BC] = build_module(BC)
    nc = _NC_CACHE[BC]

    in_maps = []
    for c in range(NCORES):
        in_maps.append(
            {
                "xT": np.ascontiguousarray(x[BC * c : BC * (c + 1)].T),
                "q": np.ascontiguousarray(q[BC * c : BC * (c + 1)]),
                "W1": w1c,
                "W2": w2,
                "W3": w3,
                "bias_all": bias,
                "m112": m112,
            }
        )

    res = bass_utils.run_bass_kernel_spmd(nc, in_maps, core_ids=list(range(NCORES)))
    LAST_EXEC_TIME_NS = res.exec_time_ns
    out = np.empty((B, HOR, QT), dtype=np.float32)
    for c in range(NCORES):
        out[BC * c : BC * (c + 1)] = np.transpose(
            res.results[c]["r_out"], (1, 0, 2)
        )
    return out


# revision 7
# speedup vs baseline: 1.0207x; 1.0207x over previous
"""Trainium2 Bass kernel for nn_MultiHeadQuantileNBEATS.

Reference computation (per batch row b):
  feats = x @ W_bb + b_bb                                   [D]
  h1[q] = relu(feats @ W1[q] + b1[q])                       [QF, H1]
  h2[q] = relu(h1[q] @ W2[q] + b2[q])                       [QF, H2]
  o3[q] = h2[q] @ W3[q] + b3[q]                             [QF, HOR]
  sq    = sort(o3 over q)  (per (b, hor))                   [HOR, QF]
  out[b, h, t] = sort_t(interp(sq[b, h, :], q[b, t]))       [HOR, QT]

Device algorithm notes:
  * Pure data parallel over 8 cores (batch sharded, weights replicated).
  * The backbone is folded into the first head layer on the HOST:
      W1c[q] = W_bb @ W1[q],  b1c[q] = b_bb @ W1[q] + b1[q]
    so the device computes h1 = relu(x @ W1c + b1c) directly.  x is
    pre-transposed on the host ([T, B_core]); no on-chip x transposes.
  * All matmuls run in ERROR-COMPENSATED FP16 (3 single-cycle-per-row
    passes instead of one fp32 matmul at 4 cycles/row):
      v = hi + lo with hi = fp16(v), lo = fp16(v - hi)
      W @ X = Whi@Xhi + Whi@Xlo + Wlo@Xhi   (+ O(2^-22) dropped term)
    accumulated exactly in the fp32 PSUM.  Weights are pre-scaled by 64
    on the host so their lo parts stay in the fp16 normal range; the
    scale is undone for free by the activation's `scale` argument.
    Measured accuracy of this scheme matches fp32 (8e-7 abs on [512]
    dot products); measured speed is 3x216ns vs 1030ns per K-chunk.
  * The final sort over the QT axis is eliminated: the quantile
    interpolant is monotone in the query level, so sorting q per row
    first yields an already-sorted output.
  * Interpolation r[b,h,t] = sum_i a_i(q[b,t]) * sq_i[b,h] is a K=112 PE
    matmul per group of 16 samples: lhsT = transposed sorted head outputs
    (fp16 hi/lo), rhs = block-diagonal coefficient matrix A (fp16 hi/lo,
    built by mask-multiply on DVE/GPSIMD).
  * Two-supertile software pipeline: PE order is heads(0), heads(1),
    interp(0), interp(1); the DVE runs the 7-way sort of supertile st
    while the PE computes heads(st+1), so the PE never idles and the HAM
    clock gate stays warm.
  * Per-core output is written fp16 feature-major [HOR, B_core, QT]; the
    host converts/transposes to [B, HOR, QT] f32 when gathering.
"""

import dataclasses
from contextlib import ExitStack

import numpy as np

import concourse.bass as bass
import concourse.mybir as mybir
import concourse.tile as tile
from concourse import bass_utils
from concourse.bass import ts
from concourse.masks import make_identity

F32 = mybir.dt.float32
FP16 = mybir.dt.float16

B, T, D = 8192, 512, 512
H1, H2, HOR = 256, 128, 96
QF, QT = 7, 32
NCORES = 8
BC = B // NCORES  # batch per core
SUB = 512         # samples per super-tile
WSCALE = 64.0     # host pre-scale on weights (undone at activation evac)
QUANTILE_LEVELS = np.array(
    [0.025, 0.1, 0.25, 0.5, 0.75, 0.9, 0.975], dtype=np.float32
)

# optimal 16-CE sorting network for 7 elements (ascending), disjoint layers
SORT7_LAYERS = [
    [(1, 2), (3, 4), (5, 6)],
    [(0, 2), (3, 5), (4, 6)],
    [(0, 1), (4, 5), (2, 6)],
    [(0, 4), (1, 5)],
    [(0, 3), (2, 5)],
    [(1, 3), (2, 4)],
    [(2, 3)],
]


def batcher_layers(n):
    """Batcher odd-even mergesort compare-exchange layers (uniform distance)."""
    layers = []
    p = 1
    while p < n:
        k = p
        while k >= 1:
            pairs = []
            j = k % p
            while j + k < n:
                for i in range(min(k, n - j - k)):
                    lo, hi = i + j, i + j + k
                    if lo // (2 * p) == hi // (2 * p):
                        pairs.append((lo, hi))
                j += 2 * k
            if pairs:
                layers.append(pairs)
            k //= 2
        p *= 2
    return layers


def coalesce2(indices):
    """Coalesce sorted ints into 2-level lattices (start, ostep, ocount, icount)."""
    idx = list(indices)
    runs = []
    i = 0
    while i < len(idx):
        cnt = 1
        while i + cnt < len(idx) and idx[i + cnt] - idx[i + cnt - 1] == 1:
            cnt += 1
        runs.append((idx[i], cnt))
        i += cnt
    groups = []
    i = 0
    while i < len(runs):
        start, cnt = runs[i]
        j = i + 1
        if j < len(runs) and runs[j][1] == cnt:
            ostep = runs[j][0] - start
            k = j
            while (
                k < len(runs)
                and runs[k][1] == cnt
                and runs[k][0] - runs[k - 1][0] == ostep
            ):
                k += 1
            groups.append((start, ostep, k - i, cnt))
            i = k
        else:
            groups.append((start, 1, 1, cnt))
            i += 1
    return groups


def _view(ap, free_dims, extra_offset):
    """Rebuild an AP keeping its partition dim, with custom free-dim lattice."""
    dims = [tuple(ap.ap[0])] + [tuple(d) for d in free_dims]
    return dataclasses.replace(ap, ap=tuple(dims), offset=ap.offset + extra_offset)


def _host_constants(b1c, b2, b3):
    # bias_all [128, 32]: packed per-partition bias columns
    bias = np.zeros((128, 32), dtype=np.float32)
    for qh in range(QF):
        for mc in range(H1 // 128):
            bias[:, 2 * qh + mc] = b1c[qh, 128 * mc : 128 * (mc + 1)]
        bias[:, 14 + qh] = b2[qh]
        bias[:96, 21 + qh] = b3[qh]
    # M112 [112, 512]: block-diagonal 0/1 mask over (sample, coeff) x (sample, t)
    m112 = np.zeros((112, 512), dtype=np.float16)
    for s in range(16):
        m112[7 * s : 7 * s + 7, 32 * s : 32 * s + 32] = 1.0
    return bias, m112


def _split16(v):
    hi = v.astype(np.float16)
    lo = (v - hi.astype(np.float32)).astype(np.float16)
    return hi, lo


# ---------------------------------------------------------------------------
# device kernel
# ---------------------------------------------------------------------------

def _emit(ctx: ExitStack, tc: tile.TileContext, ins, outs, bc=BC):
    nc = tc.nc
    (xh_d, xl_d, q_d, w1h_d, w1l_d, w2h_d, w2l_d, w3h_d, w3l_d,
     bias_d, m112_d) = ins
    (r_d,) = outs
    n_sub = bc // SUB
    n_chunk = bc // 128
    ngrp = SUB // 16
    ql = QUANTILE_LEVELS

    cpool = ctx.enter_context(tc.tile_pool(name="cpool", bufs=1))
    wpool = ctx.enter_context(tc.tile_pool(name="wpool", bufs=1))
    qpool = ctx.enter_context(tc.tile_pool(name="qpool", bufs=1))
    xpool = ctx.enter_context(tc.tile_pool(name="xpool", bufs=1))
    h1pool = ctx.enter_context(tc.tile_pool(name="h1pool", bufs=1))
    h2pool = ctx.enter_context(tc.tile_pool(name="h2pool", bufs=1))
    fscr = ctx.enter_context(tc.tile_pool(name="fscr", bufs=3))
    o3pool = ctx.enter_context(tc.tile_pool(name="o3pool", bufs=9))
    sqgpool = ctx.enter_context(tc.tile_pool(name="sqgpool", bufs=2))
    sqTpool = ctx.enter_context(tc.tile_pool(name="sqTpool", bufs=4))
    apool = ctx.enter_context(tc.tile_pool(name="apool", bufs=4))
    rpool = ctx.enter_context(tc.tile_pool(name="rpool", bufs=4))
    tpsum = ctx.enter_context(tc.tile_pool(name="tpsum", bufs=3, space="PSUM"))
    hpsum = ctx.enter_context(tc.tile_pool(name="hpsum", bufs=2, space="PSUM"))
    rpsum = ctx.enter_context(tc.tile_pool(name="rpsum", bufs=2, space="PSUM"))

    # --- constants ---
    ident = cpool.tile([128, 128], F32)
    make_identity(nc, ident[:])
    bias_sb = cpool.tile([128, 32], F32)
    nc.sync.dma_start(bias_sb[:], bias_d)
    m112 = cpool.tile([112, 512], FP16)
    nc.sync.dma_start(m112[:], m112_d)

    # PE warm-up
    warm_ps = tpsum.tile([128, 128], F32, tag="tps")
    nc.tensor.matmul(warm_ps[:], lhsT=ident[:], rhs=ident[:], start=True, stop=True)

    # --- input / weight DMAs, ordered so the PE can start early ---
    xh_sb = [[None] * (T // 128) for _ in range(n_sub)]
    xl_sb = [[None] * (T // 128) for _ in range(n_sub)]
    w1h_sb, w1l_sb = [], []
    for st in range(n_sub):
        for tci in range(T // 128):
            xh = xpool.tile([128, SUB], FP16, name=f"xh{st}_{tci}")
            nc.sync.dma_start(xh[:], xh_d[ts(tci, 128), ts(st, SUB)])
            xh_sb[st][tci] = xh
            xl = xpool.tile([128, SUB], FP16, name=f"xl{st}_{tci}")
            nc.sync.dma_start(xl[:], xl_d[ts(tci, 128), ts(st, SUB)])
            xl_sb[st][tci] = xl
        if st == 0:
            q_all = qpool.tile([128, n_chunk * QT], F32)
            nc.sync.dma_start(
                q_all[:].rearrange("p (c t) -> p c t", c=n_chunk),
                q_d.rearrange("(c p) t -> p c t", c=n_chunk),
            )
            for qh in range(QF):
                for (tag, lst, src) in (("h", w1h_sb, w1h_d),
                                        ("l", w1l_sb, w1l_d)):
                    w = wpool.tile([128, (D // 128) * H1], FP16,
                                   name=f"w1{tag}_{qh}")
                    nc.sync.dma_start(
                        w[:].rearrange("p (c m) -> p c m", c=D // 128),
                        src[qh].rearrange("(c p) m -> p c m", c=D // 128),
                    )
                    lst.append(w)

    w2h_sb, w2l_sb = [], []
    for qh in range(QF):
        for (tag, lst, src) in (("h", w2h_sb, w2h_d), ("l", w2l_sb, w2l_d)):
            w = wpool.tile([128, (H1 // 128) * H2], FP16, name=f"w2{tag}_{qh}")
            nc.sync.dma_start(
                w[:].rearrange("p (c m) -> p c m", c=H1 // 128),
                src[qh].rearrange("(c p) m -> p c m", c=H1 // 128),
            )
            lst.append(w)
    w3h_sb, w3l_sb = [], []
    for qh in range(QF):
        for (tag, lst, src) in (("h", w3h_sb, w3h_d), ("l", w3l_sb, w3l_d)):
            w = wpool.tile([128, HOR], FP16, name=f"w3{tag}_{qh}")
            nc.sync.dma_start(w[:], src[qh])
            lst.append(w)

    # =====================================================================
    # q path: per-row sort over QT on the DVE (overlaps the weight DMAs)
    # =====================================================================
    qscratch = qpool.tile([128, n_chunk * QT], F32)
    for layer in batcher_layers(QT):
        dist = layer[0][1] - layer[0][0]
        for (start, ostep, ocount, icount) in coalesce2(
            sorted(a for a, _ in layer)
        ):
            fd = [(QT, n_chunk), (ostep, ocount), (1, icount)]
            lo = _view(q_all[:], fd, start)
            hi = _view(q_all[:], fd, start + dist)
            sc = _view(qscratch[:], [(QT, n_chunk), (icount, ocount), (1, icount)], 0)
            nc.vector.tensor_tensor(sc, lo, hi, op=mybir.AluOpType.min)
            nc.vector.tensor_tensor(hi, lo, hi, op=mybir.AluOpType.max)
            nc.vector.tensor_copy(lo, sc)

    # =====================================================================
    # emission helpers
    # =====================================================================

    def emit_qT_ain(st):
        """Transpose sorted q (PE) and build ain [QT, 7*SUB] f32 on the DVE."""
        qT = qpool.tile([QT, SUB], F32, name=f"qT{st}", tag="qT", bufs=2)
        for c in range(SUB // 128):
            ps = tpsum.tile([QT, 128], F32, tag="tps")
            nc.tensor.matmul(
                ps[:], lhsT=q_all[:, ts(st * (SUB // 128) + c, QT)],
                rhs=ident[:], start=True, stop=True,
            )
            nc.scalar.copy(qT[:, ts(c, 128)], ps[:])

        ain = qpool.tile([QT, 7 * SUB], F32, name=f"ain{st}", tag="ain", bufs=2)

        def ain_view(i):
            return _view(ain[:], [(7, SUB)], i)

        f_prev = None
        for i in range(1, 7):
            lam = float(ql[i - 1])
            inv = float(
                np.float32(1.0)
                / (np.float32(ql[i] - ql[i - 1]) + np.float32(1e-8))
            )
            u = qpool.tile([QT, SUB], F32, name=f"u{st}_{i}", tag="utile", bufs=2)
            nc.vector.tensor_scalar(
                u[:], qT[:], lam, inv,
                mybir.AluOpType.subtract, mybir.AluOpType.mult,
            )
            f = qpool.tile([QT, SUB], F32, name=f"f{st}_{i}", tag=f"fp{i % 2}",
                           bufs=1)
            nc.vector.tensor_scalar(
                f[:], u[:], 1.0, 0.0, mybir.AluOpType.min, mybir.AluOpType.max
            )
            if i == 1:
                nc.vector.tensor_scalar(
                    ain_view(0), f[:], -1.0, 1.0,
                    mybir.AluOpType.mult, mybir.AluOpType.add,
                )
            else:
                nc.vector.tensor_tensor(
                    ain_view(i - 1), f_prev[:], f[:], op=mybir.AluOpType.subtract
                )
            f_prev = f
        nc.vector.tensor_copy(ain_view(6), f_prev[:])
        return ain

    def comp_mm(ps, whi, wlo, xhi, xlo, nk, first, last):
        """Emit the 3-pass compensated accumulation group over nk K-chunks.
        whi/wlo/xhi/xlo: callables chunk-index -> AP."""
        seq = (
            [("hh", c) for c in range(nk)]
            + [("hl", c) for c in range(nk)]
            + [("lh", c) for c in range(nk)]
        )
        for j, (kind, c) in enumerate(seq):
            lhs = whi(c) if kind[0] == "h" else wlo(c)
            rhs = xhi(c) if kind[1] == "h" else xlo(c)
            nc.tensor.matmul(
                ps, lhsT=lhs, rhs=rhs,
                start=(first and j == 0), stop=(last and j == len(seq) - 1),
            )

    def emit_heads(st):
        """h1 -> h2 -> o3 for one supertile, compensated fp16 on the PE."""
        h1h = [[None] * 2 for _ in range(QF)]
        h1l = [[None] * 2 for _ in range(QF)]
        for qh in range(QF):
            for mc in range(H1 // 128):
                ps = hpsum.tile([128, SUB], F32, tag="hps")
                comp_mm(
                    ps[:],
                    lambda c, qh=qh, mc=mc: w1h_sb[qh][:, ts(c * 2 + mc, 128)],
                    lambda c, qh=qh, mc=mc: w1l_sb[qh][:, ts(c * 2 + mc, 128)],
                    lambda c, st=st: xh_sb[st][c][:],
                    lambda c, st=st: xl_sb[st][c][:],
                    4, True, True,
                )
                bcol = bias_sb[:, 2 * qh + mc : 2 * qh + mc + 1]
                hh = h1pool.tile([128, SUB], FP16, name=f"h1h_{st}_{qh}_{mc}",
                                 tag=f"h1h_{qh}_{mc}", bufs=1)
                nc.scalar.activation(
                    hh[:], ps[:], mybir.ActivationFunctionType.Relu,
                    bias=bcol, scale=1.0 / WSCALE,
                )
                hf = fscr.tile([128, SUB], F32, tag="hfull")
                nc.scalar.activation(
                    hf[:], ps[:], mybir.ActivationFunctionType.Relu,
                    bias=bcol, scale=1.0 / WSCALE,
                )
                hl = h1pool.tile([128, SUB], FP16, name=f"h1l_{st}_{qh}_{mc}",
                                 tag=f"h1l_{qh}_{mc}", bufs=1)
                nc.vector.tensor_tensor(
                    hl[:], hf[:], hh[:], op=mybir.AluOpType.subtract
                )
                h1h[qh][mc] = hh
                h1l[qh][mc] = hl

        h2h = [None] * QF
        h2l = [None] * QF
        for qh in range(QF):
            ps = hpsum.tile([128, SUB], F32, tag="hps")
            for mc in range(H1 // 128):
                comp_mm(
                    ps[:],
                    lambda c, qh=qh, mc=mc: w2h_sb[qh][:, ts(mc, H2)],
                    lambda c, qh=qh, mc=mc: w2l_sb[qh][:, ts(mc, H2)],
                    lambda c, qh=qh, mc=mc: h1h[qh][mc][:],
                    lambda c, qh=qh, mc=mc: h1l[qh][mc][:],
                    1, mc == 0, mc == 1,
                )
            bcol = bias_sb[:, 14 + qh : 15 + qh]
            hh = h2pool.tile([128, SUB], FP16, name=f"h2h_{st}_{qh}",
                             tag=f"h2h_{qh}", bufs=1)
            nc.scalar.activation(
                hh[:], ps[:], mybir.ActivationFunctionType.Relu,
                bias=bcol, scale=1.0 / WSCALE,
            )
            hf = fscr.tile([128, SUB], F32, tag="hfull")
            nc.scalar.activation(
                hf[:], ps[:], mybir.ActivationFunctionType.Relu,
                bias=bcol, scale=1.0 / WSCALE,
            )
            hl = h2pool.tile([128, SUB], FP16, name=f"h2l_{st}_{qh}",
                             tag=f"h2l_{qh}", bufs=1)
            nc.vector.tensor_tensor(
                hl[:], hf[:], hh[:], op=mybir.AluOpType.subtract
            )
            h2h[qh] = hh
            h2l[qh] = hl

        o3 = [None] * QF
        for qh in range(QF):
            ps = hpsum.tile([HOR, SUB], F32, tag="hps")
            comp_mm(
                ps[:],
                lambda c, qh=qh: w3h_sb[qh][:, :],
                lambda c, qh=qh: w3l_sb[qh][:, :],
                lambda c, qh=qh: h2h[qh][:],
                lambda c, qh=qh: h2l[qh][:],
                1, True, True,
            )
            o = o3pool.tile([HOR, SUB], F32, name=f"o3_{st}_{qh}", tag="sortt")
            nc.scalar.activation(
                o[:], ps[:], mybir.ActivationFunctionType.Identity,
                bias=bias_sb[:HOR, 21 + qh : 22 + qh], scale=1.0 / WSCALE,
            )
            o3[qh] = o
        return o3

    def emit_sort(st, o3):
        """7-element sort network on the DVE; final values land interleaved
        in SQG [96, 32*112] (free idx = 112*g + 7*s + i)."""
        sqg = sqgpool.tile([HOR, ngrp * 112], F32, name=f"sqg{st}", tag="sqg")
        last_touch = {}
        for li, layer in enumerate(SORT7_LAYERS):
            for (a, b) in layer:
                last_touch[a] = (li, a, b)
                last_touch[b] = (li, a, b)
        cur = {k: o3[k] for k in range(QF)}

        def sqg_slot(j):
            return _view(sqg[:], [(112, ngrp), (7, 16)], j)

        ce_idx = 0
        for li, layer in enumerate(SORT7_LAYERS):
            for (a, b) in layer:
                ia = cur[a][:].rearrange("p (g s) -> p g s", g=ngrp)
                ib = cur[b][:].rearrange("p (g s) -> p g s", g=ngrp)
                a_final = last_touch[a] == (li, a, b)
                b_final = last_touch[b] == (li, a, b)
                if a_final:
                    oa = sqg_slot(a)
                else:
                    ta = o3pool.tile([HOR, SUB], F32, name=f"s{st}_{ce_idx}a",
                                     tag="sortt")
                    oa = ta[:].rearrange("p (g s) -> p g s", g=ngrp)
                if b_final:
                    ob = sqg_slot(b)
                else:
                    tb = o3pool.tile([HOR, SUB], F32, name=f"s{st}_{ce_idx}b",
                                     tag="sortt")
                    ob = tb[:].rearrange("p (g s) -> p g s", g=ngrp)
                nc.vector.tensor_tensor(oa, ia, ib, op=mybir.AluOpType.min)
                nc.vector.tensor_tensor(ob, ia, ib, op=mybir.AluOpType.max)
                if not a_final:
                    cur[a] = ta
                if not b_final:
                    cur[b] = tb
                ce_idx += 1
        return sqg

    def emit_interp(st, sqg, ain):
        """Per 16-sample group: PE transpose, hi/lo split of sq and a,
        A build (DVE hi / GPSIMD lo), 3-pass compensated interp matmul."""
        for g in range(ngrp):
            gg = st * ngrp + g  # global group index

            ps_t = tpsum.tile([112, 128], F32, tag="tps")
            nc.tensor.matmul(
                ps_t[:, :HOR], lhsT=sqg[:, 112 * g : 112 * (g + 1)],
                rhs=ident[:HOR, :HOR], start=True, stop=True,
            )
            nc.tensor.matmul(
                ps_t[:, HOR:128], lhsT=_view(ain[:], [(1, 112)], 112 * g),
                rhs=ident[:QT, :QT], start=True, stop=True,
            )
            # hi/lo split of the transposed (sq | a) block
            sqa_h = sqTpool.tile([112, 128], FP16, tag="sqah")
            nc.scalar.copy(sqa_h[:], ps_t[:])
            sqa_l = sqTpool.tile([112, 128], FP16, tag="sqal")
            nc.vector.tensor_tensor(
                sqa_l[:], ps_t[:], sqa_h[:], op=mybir.AluOpType.subtract
            )

            # A_hi/A_lo [112, 512] = broadcast(aT_hi/lo) * M112
            mv = m112[:].rearrange("p (s t) -> p s t", s=16)
            Ah = apool.tile([112, 512], FP16, tag="Ah")
            avh = sqa_h[:, HOR:128].unsqueeze(1).broadcast_to((112, 16, QT))
            nc.vector.tensor_tensor(
                Ah[:].rearrange("p (s t) -> p s t", s=16), avh, mv,
                op=mybir.AluOpType.mult,
            )
            Al = apool.tile([112, 512], FP16, tag="Al")
            avl = sqa_l[:, HOR:128].unsqueeze(1).broadcast_to((112, 16, QT))
            nc.gpsimd.tensor_tensor(
                Al[:].rearrange("p (s t) -> p s t", s=16), avl, mv,
                op=mybir.AluOpType.mult,
            )

            rps = rpsum.tile([HOR, 512], F32, tag="rps")
            nc.tensor.matmul(
                rps[:], lhsT=sqa_h[:, :HOR], rhs=Ah[:], start=True, stop=False
            )
            nc.tensor.matmul(
                rps[:], lhsT=sqa_h[:, :HOR], rhs=Al[:], start=False, stop=False
            )
            nc.tensor.matmul(
                rps[:], lhsT=sqa_l[:, :HOR], rhs=Ah[:], start=False, stop=True
            )

            r_sb = rpool.tile([HOR, 512], FP16, tag="rsb")
            nc.scalar.copy(r_sb[:], rps[:])
            nc.sync.dma_start(
                r_d[:, 16 * gg : 16 * (gg + 1), :],
                r_sb[:].rearrange("p (s t) -> p s t", s=16),
            )

    # =====================================================================
    # pipelined emission
    # =====================================================================
    ain = [None] * n_sub
    o3_0 = emit_heads(0)
    for st in range(n_sub):
        ain[st] = emit_qT_ain(st)
    sqg_0 = emit_sort(0, o3_0)
    o3_1 = emit_heads(1)
    sqg_1 = emit_sort(1, o3_1)
    emit_interp(0, sqg_0, ain[0])
    emit_interp(1, sqg_1, ain[1])


# Per-instruction-type sync-wait slot capacity in the walrus ISA descriptors.
_WAIT_CAPACITY = {}  # default: every type gets a single wait slot
_DRAIN_CAPACITY = {
    "EngineType.SP": 1,
    "EngineType.PE": 1,
}


def _split_waits(nc):
    """Some walrus ISA descriptors (LDWEIGHTS, DMA) have too few sync-wait
    slots for the waits Tile emits.  Move surplus waits of overflowing
    instructions onto drains inserted right before them on the same queue."""
    for fn in nc.m.functions:
        for blk in fn.blocks:
            insts = list(blk.instructions)
            out = []
            changed = False
            for ins in insts:
                si = ins.sync_info
                cap = _WAIT_CAPACITY.get(type(ins).__name__, 1)
                if si is not None and si.on_wait and len(si.on_wait) > cap:
                    waits = list(si.on_wait)
                    surplus = waits[:-cap]
                    dcap = _DRAIN_CAPACITY.get(str(ins.engine), 1)
                    di = 0
                    while surplus:
                        chunk, surplus = surplus[:dcap], surplus[dcap:]
                        out.append(
                            mybir.InstDrain(
                                name=f"{ins.name}-wfence{di}",
                                engine=ins.engine,
                                ins=[],
                                outs=[],
                                sync_info=mybir.SyncInfo(
                                    on_wait=chunk, on_update=[]
                                ),
                            )
                        )
                        di += 1
                    si.on_wait = waits[-cap:]
                    changed = True
                out.append(ins)
            if changed:
                blk.instructions = out


def build_module(bc=BC):
    nc = bass.Bass("TRN2", target_bir_lowering=False, debug=False)
    xh_d = nc.dram_tensor("xT_hi", [T, bc], FP16, kind="ExternalInput").ap()
    xl_d = nc.dram_tensor("xT_lo", [T, bc], FP16, kind="ExternalInput").ap()
    q_d = nc.dram_tensor("q", [bc, QT], F32, kind="ExternalInput").ap()
    w1h_d = nc.dram_tensor("W1hi", [QF, D, H1], FP16, kind="ExternalInput").ap()
    w1l_d = nc.dram_tensor("W1lo", [QF, D, H1], FP16, kind="ExternalInput").ap()
    w2h_d = nc.dram_tensor("W2hi", [QF, H1, H2], FP16, kind="ExternalInput").ap()
    w2l_d = nc.dram_tensor("W2lo", [QF, H1, H2], FP16, kind="ExternalInput").ap()
    w3h_d = nc.dram_tensor("W3hi", [QF, H2, HOR], FP16, kind="ExternalInput").ap()
    w3l_d = nc.dram_tensor("W3lo", [QF, H2, HOR], FP16, kind="ExternalInput").ap()
    bias_d = nc.dram_tensor("bias_all", [128, 32], F32, kind="ExternalInput").ap()
    m112_d = nc.dram_tensor("m112", [112, 512], FP16, kind="ExternalInput").ap()
    r_d = nc.dram_tensor("r_out", [HOR, bc, QT], FP16, kind="ExternalOutput").ap()

    with tile.TileContext(nc) as tc:
        with ExitStack() as ctx:
            _emit(ctx, tc,
                  (xh_d, xl_d, q_d, w1h_d, w1l_d, w2h_d, w2l_d, w3h_d, w3l_d,
                   bias_d, m112_d),
                  (r_d,), bc=bc)
    _split_waits(nc)
    return nc


_NC_CACHE = {}
LAST_EXEC_TIME_NS = None


def kernel(**inputs) -> np.ndarray:
    global LAST_EXEC_TIME_NS
    x = np.asarray(inputs["x"], dtype=np.float32)
    q = np.asarray(inputs["q"], dtype=np.float32)
    w_bb = np.asarray(inputs["W_bb"], dtype=np.float64)
    b_bb = np.asarray(inputs["b_bb"], dtype=np.float64)
    w1 = np.asarray(inputs["W1"], dtype=np.float64)
    b1 = np.asarray(inputs["b1"], dtype=np.float64)
    w2 = np.asarray(inputs["W2"], dtype=np.float32)
    w3 = np.asarray(inputs["W3"], dtype=np.float32)

    # Fold the backbone into the first head layer (float64 on the host).
    w1c = (w_bb[None, :, :] @ w1).astype(np.float32)
    b1c = np.ascontiguousarray((b_bb @ w1 + b1).astype(np.float32))

    w1hi, w1lo = _split16(w1c * WSCALE)
    w2hi, w2lo = _split16(w2 * WSCALE)
    w3hi, w3lo = _split16(w3 * WSCALE)

    bias, m112 = _host_constants(
        b1c,
        np.asarray(inputs["b2"], dtype=np.float32),
        np.asarray(inputs["b3"], dtype=np.float32),
    )

    if BC not in _NC_CACHE:
        _NC_CACHE[BC] = build_module(BC)
    nc = _NC_CACHE[BC]

    in_maps = []
    for c in range(NCORES):
        xT = np.ascontiguousarray(x[BC * c : BC * (c + 1)].T)
        xhi, xlo = _split16(xT)
        in_maps.append(
            {
                "xT_hi": xhi,
                "xT_lo": xlo,
                "q": np.ascontiguousarray(q[BC * c : BC * (c + 1)]),
                "W1hi": w1hi, "W1lo": w1lo,
                "W2hi": w2hi, "W2lo": w2lo,
                "W3hi": w3hi, "W3lo": w3lo,
                "bias_all": bias,
                "m112": m112,
            }
        )

    res = bass_utils.run_bass_kernel_spmd(nc, in_maps, core_ids=list(range(NCORES)))
    LAST_EXEC_TIME_NS = res.exec_time_ns
    out = np.empty((B, HOR, QT), dtype=np.float32)
    for c in range(NCORES):
        out[BC * c : BC * (c + 1)] = np.transpose(
            res.results[c]["r_out"].astype(np.float32), (1, 0, 2)
        )
    return out


# revision 12
# speedup vs baseline: 1.2741x; 1.2482x over previous
"""Trainium2 Bass kernel for nn_MultiHeadQuantileNBEATS.

Reference computation (per batch row b):
  feats = x @ W_bb + b_bb                                   [D]
  h1[q] = relu(feats @ W1[q] + b1[q])                       [QF, H1]
  h2[q] = relu(h1[q] @ W2[q] + b2[q])                       [QF, H2]
  o3[q] = h2[q] @ W3[q] + b3[q]                             [QF, HOR]
  sq    = sort(o3 over q)  (per (b, hor))                   [HOR, QF]
  out[b, h, t] = sort_t(interp(sq[b, h, :], q[b, t]))       [HOR, QT]

Device algorithm notes:
  * Pure data parallel over 8 cores (batch sharded, weights replicated).
  * The backbone is folded into the first head layer on the HOST:
      W1c[q] = W_bb @ W1[q],  b1c[q] = b_bb @ W1[q] + b1[q]
    so the device computes h1 = relu(x @ W1c + b1c) directly.  x is
    pre-transposed on the host ([T, B_core]); no on-chip x transposes.
  * All matmuls run in ERROR-COMPENSATED FP16 (3 single-cycle-per-row
    passes instead of one fp32 matmul at 4 cycles/row):
      v = hi + lo with hi = fp16(v), lo = fp16(v - hi)
      W @ X = Whi@Xhi + Whi@Xlo + Wlo@Xhi   (+ O(2^-22) dropped term)
    accumulated exactly in the fp32 PSUM.  Weights are pre-scaled by 64
    on the host so their lo parts stay in the fp16 normal range; the
    scale is undone for free by the activation's `scale` argument.
    Measured accuracy of this scheme matches fp32 (8e-7 abs on [512]
    dot products); measured speed is 3x216ns vs 1030ns per K-chunk.
  * The final sort over the QT axis is eliminated: the quantile
    interpolant is monotone in the query level, so sorting q per row
    first yields an already-sorted output.
  * Interpolation r[b,h,t] = sum_i a_i(q[b,t]) * sq_i[b,h] is a K=112 PE
    matmul per group of 16 samples: lhsT = transposed sorted head outputs
    (fp16 hi/lo), rhs = block-diagonal coefficient matrix A (fp16 hi/lo,
    built by mask-multiply on DVE/GPSIMD).
  * Two-supertile software pipeline: PE order is heads(0), heads(1),
    interp(0), interp(1); the DVE runs the 7-way sort of supertile st
    while the PE computes heads(st+1), so the PE never idles and the HAM
    clock gate stays warm.
  * Per-core output is written fp16 feature-major [HOR, B_core, QT]; the
    host converts/transposes to [B, HOR, QT] f32 when gathering.
"""

import dataclasses
from contextlib import ExitStack

import numpy as np

import concourse.bass as bass
import concourse.mybir as mybir
import concourse.tile as tile
from concourse import bass_utils
from concourse.bass import ts
from concourse.masks import make_identity

F32 = mybir.dt.float32
FP16 = mybir.dt.float16

B, T, D = 8192, 512, 512
H1, H2, HOR = 256, 128, 96
QF, QT = 7, 32
NCORES = 8
BC = B // NCORES  # batch per core
SUB = 512         # samples per super-tile
WSCALE = 64.0     # host pre-scale on weights (undone at activation evac)
QUANTILE_LEVELS = np.array(
    [0.025, 0.1, 0.25, 0.5, 0.75, 0.9, 0.975], dtype=np.float32
)

# optimal 16-CE sorting network for 7 elements (ascending), disjoint layers
SORT7_LAYERS = [
    [(1, 2), (3, 4), (5, 6)],
    [(0, 2), (3, 5), (4, 6)],
    [(0, 1), (4, 5), (2, 6)],
    [(0, 4), (1, 5)],
    [(0, 3), (2, 5)],
    [(1, 3), (2, 4)],
    [(2, 3)],
]


def batcher_layers(n):
    """Batcher odd-even mergesort compare-exchange layers (uniform distance)."""
    layers = []
    p = 1
    while p < n:
        k = p
        while k >= 1:
            pairs = []
            j = k % p
            while j + k < n:
                for i in range(min(k, n - j - k)):
                    lo, hi = i + j, i + j + k
                    if lo // (2 * p) == hi // (2 * p):
                        pairs.append((lo, hi))
                j += 2 * k
            if pairs:
                layers.append(pairs)
            k //= 2
        p *= 2
    return layers


def coalesce2(indices):
    """Coalesce sorted ints into 2-level lattices (start, ostep, ocount, icount)."""
    idx = list(indices)
    runs = []
    i = 0
    while i < len(idx):
        cnt = 1
        while i + cnt < len(idx) and idx[i + cnt] - idx[i + cnt - 1] == 1:
            cnt += 1
        runs.append((idx[i], cnt))
        i += cnt
    groups = []
    i = 0
    while i < len(runs):
        start, cnt = runs[i]
        j = i + 1
        if j < len(runs) and runs[j][1] == cnt:
            ostep = runs[j][0] - start
            k = j
            while (
                k < len(runs)
                and runs[k][1] == cnt
                and runs[k][0] - runs[k - 1][0] == ostep
            ):
                k += 1
            groups.append((start, ostep, k - i, cnt))
            i = k
        else:
            groups.append((start, 1, 1, cnt))
            i += 1
    return groups


def _view(ap, free_dims, extra_offset):
    """Rebuild an AP keeping its partition dim, with custom free-dim lattice."""
    dims = [tuple(ap.ap[0])] + [tuple(d) for d in free_dims]
    return dataclasses.replace(ap, ap=tuple(dims), offset=ap.offset + extra_offset)


def _host_constants(b1c, b2, b3):
    # bias_all [128, 32]: packed per-partition bias columns
    bias = np.zeros((128, 32), dtype=np.float32)
    for qh in range(QF):
        for mc in range(H1 // 128):
            bias[:, 2 * qh + mc] = b1c[qh, 128 * mc : 128 * (mc + 1)]
        bias[:, 14 + qh] = b2[qh]
        bias[:96, 21 + qh] = b3[qh]
    # M112 [112, 512]: block-diagonal 0/1 mask over (sample, coeff) x (sample, t)
    m112 = np.zeros((112, 512), dtype=np.float32)
    for s in range(16):
        m112[7 * s : 7 * s + 7, 32 * s : 32 * s + 32] = 1.0
    return bias, m112


def _split16(v):
    hi = v.astype(np.float16)
    lo = (v - hi.astype(np.float32)).astype(np.float16)
    return hi, lo


# ---------------------------------------------------------------------------
# device kernel
# ---------------------------------------------------------------------------

def _emit(ctx: ExitStack, tc: tile.TileContext, ins, outs, bc=BC):
    nc = tc.nc
    (xh_d, xl_d, q_d, w1h_d, w1l_d, w2h_d, w2l_d, w3h_d, w3l_d,
     bias_d, m112_d) = ins
    (r_d,) = outs
    n_sub = bc // SUB
    n_chunk = bc // 128
    ngrp = SUB // 16
    ql = QUANTILE_LEVELS

    cpool = ctx.enter_context(tc.tile_pool(name="cpool", bufs=1))
    wpool = ctx.enter_context(tc.tile_pool(name="wpool", bufs=1))
    qpool = ctx.enter_context(tc.tile_pool(name="qpool", bufs=1))
    xpool = ctx.enter_context(tc.tile_pool(name="xpool", bufs=1))
    h1pool = ctx.enter_context(tc.tile_pool(name="h1pool", bufs=1))
    h2pool = ctx.enter_context(tc.tile_pool(name="h2pool", bufs=1))
    fscr = ctx.enter_context(tc.tile_pool(name="fscr", bufs=3))
    o3pool = ctx.enter_context(tc.tile_pool(name="o3pool", bufs=9))
    sqgpool = ctx.enter_context(tc.tile_pool(name="sqgpool", bufs=2))
    sqTpool = ctx.enter_context(tc.tile_pool(name="sqTpool", bufs=4))
    apool = ctx.enter_context(tc.tile_pool(name="apool", bufs=4))
    rpool = ctx.enter_context(tc.tile_pool(name="rpool", bufs=4))
    tpsum = ctx.enter_context(tc.tile_pool(name="tpsum", bufs=3, space="PSUM"))
    hpsum = ctx.enter_context(tc.tile_pool(name="hpsum", bufs=2, space="PSUM"))
    rpsum = ctx.enter_context(tc.tile_pool(name="rpsum", bufs=2, space="PSUM"))

    # --- constants ---
    ident = cpool.tile([128, 128], F32)
    make_identity(nc, ident[:])
    bias_sb = cpool.tile([128, 32], F32)
    nc.sync.dma_start(bias_sb[:], bias_d)
    m112 = cpool.tile([112, 512], F32)
    nc.sync.dma_start(m112[:], m112_d)

    # PE warm-up
    warm_ps = tpsum.tile([128, 128], F32, tag="tps")
    nc.tensor.matmul(warm_ps[:], lhsT=ident[:], rhs=ident[:], start=True, stop=True)

    # --- input / weight DMAs, ordered so the PE can start early ---
    xh_sb = [[None] * (T // 128) for _ in range(n_sub)]
    xl_sb = [[None] * (T // 128) for _ in range(n_sub)]
    w1h_sb, w1l_sb = [], []
    for st in range(n_sub):
        for tci in range(T // 128):
            xh = xpool.tile([128, SUB], FP16, name=f"xh{st}_{tci}")
            nc.sync.dma_start(xh[:], xh_d[ts(tci, 128), ts(st, SUB)])
            xh_sb[st][tci] = xh
            xl = xpool.tile([128, SUB], FP16, name=f"xl{st}_{tci}")
            nc.sync.dma_start(xl[:], xl_d[ts(tci, 128), ts(st, SUB)])
            xl_sb[st][tci] = xl
        if st == 0:
            q_all = qpool.tile([128, n_chunk * QT], F32)
            nc.sync.dma_start(
                q_all[:].rearrange("p (c t) -> p c t", c=n_chunk),
                q_d.rearrange("(c p) t -> p c t", c=n_chunk),
            )
            for qh in range(QF):
                for (tag, lst, src) in (("h", w1h_sb, w1h_d),
                                        ("l", w1l_sb, w1l_d)):
                    w = wpool.tile([128, (D // 128) * H1], FP16,
                                   name=f"w1{tag}_{qh}")
                    nc.sync.dma_start(
                        w[:].rearrange("p (c m) -> p c m", c=D // 128),
                        src[qh].rearrange("(c p) m -> p c m", c=D // 128),
                    )
                    lst.append(w)

    w2h_sb, w2l_sb = [], []
    for qh in range(QF):
        for (tag, lst, src) in (("h", w2h_sb, w2h_d), ("l", w2l_sb, w2l_d)):
            w = wpool.tile([128, (H1 // 128) * H2], FP16, name=f"w2{tag}_{qh}")
            nc.sync.dma_start(
                w[:].rearrange("p (c m) -> p c m", c=H1 // 128),
                src[qh].rearrange("(c p) m -> p c m", c=H1 // 128),
            )
            lst.append(w)
    w3h_sb, w3l_sb = [], []
    for qh in range(QF):
        for (tag, lst, src) in (("h", w3h_sb, w3h_d), ("l", w3l_sb, w3l_d)):
            w = wpool.tile([128, HOR], FP16, name=f"w3{tag}_{qh}")
            nc.sync.dma_start(w[:], src[qh])
            lst.append(w)

    # =====================================================================
    # q path: per-row sort over QT on the DVE (overlaps the weight DMAs)
    # =====================================================================
    qscratch = qpool.tile([128, n_chunk * QT], F32)
    for layer in batcher_layers(QT):
        dist = layer[0][1] - layer[0][0]
        for (start, ostep, ocount, icount) in coalesce2(
            sorted(a for a, _ in layer)
        ):
            fd = [(QT, n_chunk), (ostep, ocount), (1, icount)]
            lo = _view(q_all[:], fd, start)
            hi = _view(q_all[:], fd, start + dist)
            sc = _view(qscratch[:], [(QT, n_chunk), (icount, ocount), (1, icount)], 0)
            nc.vector.tensor_tensor(sc, lo, hi, op=mybir.AluOpType.min)
            nc.vector.tensor_tensor(hi, lo, hi, op=mybir.AluOpType.max)
            nc.vector.tensor_copy(lo, sc)

    # =====================================================================
    # emission helpers
    # =====================================================================

    def emit_qT_ain(st):
        """Transpose sorted q (PE) and build ain [QT, 7*SUB] f32 on the DVE."""
        qT = qpool.tile([QT, SUB], F32, name=f"qT{st}", tag="qT", bufs=2)
        for c in range(SUB // 128):
            ps = tpsum.tile([QT, 128], F32, tag="tps")
            nc.tensor.matmul(
                ps[:], lhsT=q_all[:, ts(st * (SUB // 128) + c, QT)],
                rhs=ident[:], start=True, stop=True,
            )
            nc.scalar.copy(qT[:, ts(c, 128)], ps[:])

        ain = qpool.tile([QT, 7 * SUB], F32, name=f"ain{st}", tag="ain", bufs=2)

        def ain_view(i):
            return _view(ain[:], [(7, SUB)], i)

        f_prev = None
        for i in range(1, 7):
            lam = float(ql[i - 1])
            inv = float(
                np.float32(1.0)
                / (np.float32(ql[i] - ql[i - 1]) + np.float32(1e-8))
            )
            u = qpool.tile([QT, SUB], F32, name=f"u{st}_{i}", tag="utile", bufs=2)
            nc.vector.tensor_scalar(
                u[:], qT[:], lam, inv,
                mybir.AluOpType.subtract, mybir.AluOpType.mult,
            )
            f = qpool.tile([QT, SUB], F32, name=f"f{st}_{i}", tag=f"fp{i % 2}",
                           bufs=1)
            nc.vector.tensor_scalar(
                f[:], u[:], 1.0, 0.0, mybir.AluOpType.min, mybir.AluOpType.max
            )
            if i == 1:
                nc.vector.tensor_scalar(
                    ain_view(0), f[:], -1.0, 1.0,
                    mybir.AluOpType.mult, mybir.AluOpType.add,
                )
            else:
                nc.vector.tensor_tensor(
                    ain_view(i - 1), f_prev[:], f[:], op=mybir.AluOpType.subtract
                )
            f_prev = f
        nc.vector.tensor_copy(ain_view(6), f_prev[:])
        return ain

    def comp_mm(ps, whi, wlo, xhi, xlo, nk, first, last):
        """Emit the 3-pass compensated accumulation group over nk K-chunks.
        whi/wlo/xhi/xlo: callables chunk-index -> AP."""
        seq = (
            [("hh", c) for c in range(nk)]
            + [("hl", c) for c in range(nk)]
            + [("lh", c) for c in range(nk)]
        )
        for j, (kind, c) in enumerate(seq):
            lhs = whi(c) if kind[0] == "h" else wlo(c)
            rhs = xhi(c) if kind[1] == "h" else xlo(c)
            nc.tensor.matmul(
                ps, lhsT=lhs, rhs=rhs,
                start=(first and j == 0), stop=(last and j == len(seq) - 1),
            )

    def emit_heads(st):
        """h1 -> h2 -> o3 for one supertile, compensated fp16 on the PE."""
        h1h = [[None] * 2 for _ in range(QF)]
        h1l = [[None] * 2 for _ in range(QF)]
        for qh in range(QF):
            for mc in range(H1 // 128):
                ps = hpsum.tile([128, SUB], F32, tag="hps")
                comp_mm(
                    ps[:],
                    lambda c, qh=qh, mc=mc: w1h_sb[qh][:, ts(c * 2 + mc, 128)],
                    lambda c, qh=qh, mc=mc: w1l_sb[qh][:, ts(c * 2 + mc, 128)],
                    lambda c, st=st: xh_sb[st][c][:],
                    lambda c, st=st: xl_sb[st][c][:],
                    4, True, True,
                )
                bcol = bias_sb[:, 2 * qh + mc : 2 * qh + mc + 1]
                hh = h1pool.tile([128, SUB], FP16, name=f"h1h_{st}_{qh}_{mc}",
                                 tag=f"h1h_{qh}_{mc}", bufs=1)
                nc.scalar.activation(
                    hh[:], ps[:], mybir.ActivationFunctionType.Relu,
                    bias=bcol, scale=1.0 / WSCALE,
                )
                hf = fscr.tile([128, SUB], F32, tag="hfull")
                nc.scalar.activation(
                    hf[:], ps[:], mybir.ActivationFunctionType.Relu,
                    bias=bcol, scale=1.0 / WSCALE,
                )
                hl = h1pool.tile([128, SUB], FP16, name=f"h1l_{st}_{qh}_{mc}",
                                 tag=f"h1l_{qh}_{mc}", bufs=1)
                nc.vector.tensor_tensor(
                    hl[:], hf[:], hh[:], op=mybir.AluOpType.subtract
                )
                h1h[qh][mc] = hh
                h1l[qh][mc] = hl
        del hf, hl, hh

        h2h = [None] * QF
        h2l = [None] * QF
        for qh in range(QF):
            ps = hpsum.tile([128, SUB], F32, tag="hps")
            for mc in range(H1 // 128):
                comp_mm(
                    ps[:],
                    lambda c, qh=qh, mc=mc: w2h_sb[qh][:, ts(mc, H2)],
                    lambda c, qh=qh, mc=mc: w2l_sb[qh][:, ts(mc, H2)],
                    lambda c, qh=qh, mc=mc: h1h[qh][mc][:],
                    lambda c, qh=qh, mc=mc: h1l[qh][mc][:],
                    1, mc == 0, mc == 1,
                )
            bcol = bias_sb[:, 14 + qh : 15 + qh]
            hh = h2pool.tile([128, SUB], FP16, name=f"h2h_{st}_{qh}",
                             tag=f"h2h_{qh}", bufs=1)
            nc.scalar.activation(
                hh[:], ps[:], mybir.ActivationFunctionType.Relu,
                bias=bcol, scale=1.0 / WSCALE,
            )
            hf = fscr.tile([128, SUB], F32, tag="hfull")
            nc.scalar.activation(
                hf[:], ps[:], mybir.ActivationFunctionType.Relu,
                bias=bcol, scale=1.0 / WSCALE,
            )
            hl = h2pool.tile([128, SUB], FP16, name=f"h2l_{st}_{qh}",
                             tag=f"h2l_{qh}", bufs=1)
            nc.gpsimd.tensor_tensor(
                hl[:], hf[:], hh[:], op=mybir.AluOpType.subtract
            )
            h2h[qh] = hh
            h2l[qh] = hl

        o3 = [None] * QF
        for qh in range(QF):
            ps = hpsum.tile([HOR, SUB], F32, tag="hps")
            comp_mm(
                ps[:],
                lambda c, qh=qh: w3h_sb[qh][:, :],
                lambda c, qh=qh: w3l_sb[qh][:, :],
                lambda c, qh=qh: h2h[qh][:],
                lambda c, qh=qh: h2l[qh][:],
                1, True, True,
            )
            o = o3pool.tile([HOR, SUB], F32, name=f"o3_{st}_{qh}", tag="sortt")
            nc.scalar.activation(
                o[:], ps[:], mybir.ActivationFunctionType.Identity,
                bias=bias_sb[:HOR, 21 + qh : 22 + qh], scale=1.0 / WSCALE,
            )
            o3[qh] = o
        return o3

    def emit_sort(st, o3):
        """7-element sort network on the DVE; final values land interleaved
        in SQG [96, 32*112] (free idx = 112*g + 7*s + i)."""
        sqg = sqgpool.tile([HOR, ngrp * 112], F32, name=f"sqg{st}", tag="sqg")
        last_touch = {}
        for li, layer in enumerate(SORT7_LAYERS):
            for (a, b) in layer:
                last_touch[a] = (li, a, b)
                last_touch[b] = (li, a, b)
        cur = {k: o3[k] for k in range(QF)}

        def sqg_slot(j):
            return _view(sqg[:], [(112, ngrp), (7, 16)], j)

        ce_idx = 0
        for li, layer in enumerate(SORT7_LAYERS):
            for (a, b) in layer:
                ia = cur[a][:].rearrange("p (g s) -> p g s", g=ngrp)
                ib = cur[b][:].rearrange("p (g s) -> p g s", g=ngrp)
                a_final = last_touch[a] == (li, a, b)
                b_final = last_touch[b] == (li, a, b)
                if a_final:
                    oa = sqg_slot(a)
                else:
                    ta = o3pool.tile([HOR, SUB], F32, name=f"s{st}_{ce_idx}a",
                                     tag="sortt")
                    oa = ta[:].rearrange("p (g s) -> p g s", g=ngrp)
                if b_final:
                    ob = sqg_slot(b)
                else:
                    tb = o3pool.tile([HOR, SUB], F32, name=f"s{st}_{ce_idx}b",
                                     tag="sortt")
                    ob = tb[:].rearrange("p (g s) -> p g s", g=ngrp)
                nc.vector.tensor_tensor(oa, ia, ib, op=mybir.AluOpType.min)
                nc.vector.tensor_tensor(ob, ia, ib, op=mybir.AluOpType.max)
                if not a_final:
                    cur[a] = ta
                if not b_final:
                    cur[b] = tb
                ce_idx += 1
        return sqg

    def emit_interp(st, sqg, ain):
        """Per 16-sample group: PE transposes into one psum tile, scalar
        evacuation, A build and r evacuation alternating DVE/GPSIMD, one
        fp32 interp matmul."""
        for g in range(ngrp):
            gg = st * ngrp + g  # global group index

            ps_t = tpsum.tile([112, 128], F32, tag="tps")
            nc.tensor.matmul(
                ps_t[:, :HOR], lhsT=sqg[:, 112 * g : 112 * (g + 1)],
                rhs=ident[:HOR, :HOR], start=True, stop=True,
            )
            nc.tensor.matmul(
                ps_t[:, HOR:128], lhsT=_view(ain[:], [(1, 112)], 112 * g),
                rhs=ident[:QT, :QT], start=True, stop=True,
            )
            sqa = sqTpool.tile([112, 128], F32, tag="sqa")
            nc.scalar.copy(sqa[:], ps_t[:])

            # A [112, 512] = broadcast(aT) * M112, alternating DVE/GPSIMD
            A = apool.tile([112, 512], F32, tag="A")
            av = sqa[:, HOR:128].unsqueeze(1).broadcast_to((112, 16, QT))
            mv = m112[:].rearrange("p (s t) -> p s t", s=16)
            Av = A[:].rearrange("p (s t) -> p s t", s=16)
            eng = nc.vector if g % 2 == 0 else nc.gpsimd
            eng.tensor_tensor(Av, av, mv, op=mybir.AluOpType.mult)

            rps = rpsum.tile([HOR, 512], F32, tag="rps")
            nc.tensor.matmul(
                rps[:], lhsT=sqa[:, :HOR], rhs=A[:], start=True, stop=True
            )

            r_sb = rpool.tile([HOR, 512], FP16, tag="rsb")
            if g % 2 == 0:
                nc.scalar.copy(r_sb[:], rps[:])
            else:
                nc.vector.tensor_copy(r_sb[:], rps[:])
            nc.sync.dma_start(
                r_d[:, 16 * gg : 16 * (gg + 1), :],
                r_sb[:].rearrange("p (s t) -> p s t", s=16),
            )

    # =====================================================================
    # pipelined emission
    # =====================================================================
    ain = [None] * n_sub
    o3_0 = emit_heads(0)
    for st in range(n_sub):
        ain[st] = emit_qT_ain(st)
    sqg_0 = emit_sort(0, o3_0)
    o3_1 = emit_heads(1)
    sqg_1 = emit_sort(1, o3_1)
    emit_interp(0, sqg_0, ain[0])
    emit_interp(1, sqg_1, ain[1])


# Per-instruction-type sync-wait slot capacity in the walrus ISA descriptors.
_WAIT_CAPACITY = {}  # default: every type gets a single wait slot
_DRAIN_CAPACITY = {
    "EngineType.SP": 1,
    "EngineType.PE": 1,
}


def _split_waits(nc):
    """Some walrus ISA descriptors (LDWEIGHTS, DMA) have too few sync-wait
    slots for the waits Tile emits.  Move surplus waits of overflowing
    instructions onto drains inserted right before them on the same queue."""
    for fn in nc.m.functions:
        for blk in fn.blocks:
            insts = list(blk.instructions)
            out = []
            changed = False
            for ins in insts:
                si = ins.sync_info
                cap = _WAIT_CAPACITY.get(type(ins).__name__, 1)
                if si is not None and si.on_wait and len(si.on_wait) > cap:
                    waits = list(si.on_wait)
                    surplus = waits[:-cap]
                    dcap = _DRAIN_CAPACITY.get(str(ins.engine), 1)
                    di = 0
                    while surplus:
                        chunk, surplus = surplus[:dcap], surplus[dcap:]
                        out.append(
                            mybir.InstDrain(
                                name=f"{ins.name}-wfence{di}",
                                engine=ins.engine,
                                ins=[],
                                outs=[],
                                sync_info=mybir.SyncInfo(
                                    on_wait=chunk, on_update=[]
                                ),
                            )
                        )
                        di += 1
                    si.on_wait = waits[-cap:]
                    changed = True
                out.append(ins)
            if changed:
                blk.instructions = out


def build_module(bc=BC):
    nc = bass.Bass("TRN2", target_bir_lowering=False, debug=False)
    xh_d = nc.dram_tensor("xT_hi", [T, bc], FP16, kind="ExternalInput").ap()
    xl_d = nc.dram_tensor("xT_lo", [T, bc], FP16, kind="ExternalInput").ap()
    q_d = nc.dram_tensor("q", [bc, QT], F32, kind="ExternalInput").ap()
    w1h_d = nc.dram_tensor("W1hi", [QF, D, H1], FP16, kind="ExternalInput").ap()
    w1l_d = nc.dram_tensor("W1lo", [QF, D, H1], FP16, kind="ExternalInput").ap()
    w2h_d = nc.dram_tensor("W2hi", [QF, H1, H2], FP16, kind="ExternalInput").ap()
    w2l_d = nc.dram_tensor("W2lo", [QF, H1, H2], FP16, kind="ExternalInput").ap()
    w3h_d = nc.dram_tensor("W3hi", [QF, H2, HOR], FP16, kind="ExternalInput").ap()
    w3l_d = nc.dram_tensor("W3lo", [QF, H2, HOR], FP16, kind="ExternalInput").ap()
    bias_d = nc.dram_tensor("bias_all", [128, 32], F32, kind="ExternalInput").ap()
    m112_d = nc.dram_tensor("m112", [112, 512], F32, kind="ExternalInput").ap()
    r_d = nc.dram_tensor("r_out", [HOR, bc, QT], FP16, kind="ExternalOutput").ap()

    with tile.TileContext(nc) as tc:
        with ExitStack() as ctx:
            _emit(ctx, tc,
                  (xh_d, xl_d, q_d, w1h_d, w1l_d, w2h_d, w2l_d, w3h_d, w3l_d,
                   bias_d, m112_d),
                  (r_d,), bc=bc)
    _split_waits(nc)
    return nc


_NC_CACHE = {}
LAST_EXEC_TIME_NS = None


def kernel(**inputs) -> np.ndarray:
    global LAST_EXEC_TIME_NS
    x = np.asarray(inputs["x"], dtype=np.float32)
    q = np.asarray(inputs["q"], dtype=np.float32)
    w_bb = np.asarray(inputs["W_bb"], dtype=np.float64)
    b_bb = np.asarray(inputs["b_bb"], dtype=np.float64)
    w1 = np.asarray(inputs["W1"], dtype=np.float64)
    b1 = np.asarray(inputs["b1"], dtype=np.float64)
    w2 = np.asarray(inputs["W2"], dtype=np.float32)
    w3 = np.asarray(inputs["W3"], dtype=np.float32)

    # Fold the backbone into the first head layer (float64 on the host).
    w1c = (w_bb[None, :, :] @ w1).astype(np.float32)
    b1c = np.ascontiguousarray((b_bb @ w1 + b1).astype(np.float32))

    w1hi, w1lo = _split16(w1c * WSCALE)
    w2hi, w2lo = _split16(w2 * WSCALE)
    w3hi, w3lo = _split16(w3 * WSCALE)

    bias, m112 = _host_constants(
        b1c,
        np.asarray(inputs["b2"], dtype=np.float32),
        np.asarray(inputs["b3"], dtype=np.float32),
    )

    if BC not in _NC_CACHE:
        _NC_CACHE[BC] = build_module(BC)
    nc = _NC_CACHE[BC]

    in_maps = []
    for c in range(NCORES):
        xT = np.ascontiguousarray(x[BC * c : BC * (c + 1)].T)
        xhi, xlo = _split16(xT)
        in_maps.append(
            {
                "xT_hi": xhi,
                "xT_lo": xlo,
                "q": np.ascontiguousarray(q[BC * c : BC * (c + 1)]),
                "W1hi": w1hi, "W1lo": w1lo,
                "W2hi": w2hi, "W2lo": w2lo,
                "W3hi": w3hi, "W3lo": w3lo,
                "bias_all": bias,
                "m112": m112,
            }
        )

    res = bass_utils.run_bass_kernel_spmd(nc, in_maps, core_ids=list(range(NCORES)))
    LAST_EXEC_TIME_NS = res.exec_time_ns
    out = np.empty((B, HOR, QT), dtype=np.float32)
    for c in range(NCORES):
        out[BC * c : BC * (c + 1)] = np.transpose(
            res.results[c]["r_out"].astype(np.float32), (1, 0, 2)
        )
    return out


# revision 17
# speedup vs baseline: 1.3726x; 1.0773x over previous
"""Trainium2 Bass kernel for nn_MultiHeadQuantileNBEATS.

Reference computation (per batch row b):
  feats = x @ W_bb + b_bb                                   [D]
  h1[q] = relu(feats @ W1[q] + b1[q])                       [QF, H1]
  h2[q] = relu(h1[q] @ W2[q] + b2[q])                       [QF, H2]
  o3[q] = h2[q] @ W3[q] + b3[q]                             [QF, HOR]
  sq    = sort(o3 over q)  (per (b, hor))                   [HOR, QF]
  out[b, h, t] = sort_t(interp(sq[b, h, :], q[b, t]))       [HOR, QT]

Device algorithm notes:
  * Pure data parallel over 8 cores (batch sharded, weights replicated).
  * The backbone is folded into the first head layer on the HOST:
      W1c[q] = W_bb @ W1[q],  b1c[q] = b_bb @ W1[q] + b1[q]
    so the device computes h1 = relu(x @ W1c + b1c) directly.  x is
    pre-transposed on the host ([T, B_core]); no on-chip x transposes.
  * All matmuls run in ERROR-COMPENSATED FP16 (3 single-cycle-per-row
    passes instead of one fp32 matmul at 4 cycles/row):
      v = hi + lo with hi = fp16(v), lo = fp16(v - hi)
      W @ X = Whi@Xhi + Whi@Xlo + Wlo@Xhi   (+ O(2^-22) dropped term)
    accumulated exactly in the fp32 PSUM.  Weights are pre-scaled by 64
    on the host so their lo parts stay in the fp16 normal range; the
    scale is undone for free by the activation's `scale` argument.
    Measured accuracy of this scheme matches fp32 (8e-7 abs on [512]
    dot products); measured speed is 3x216ns vs 1030ns per K-chunk.
  * The final sort over the QT axis is eliminated: the quantile
    interpolant is monotone in the query level, so sorting q per row
    first yields an already-sorted output.
  * Interpolation r[b,h,t] = sum_i a_i(q[b,t]) * sq_i[b,h] is a K=112 PE
    matmul per group of 16 samples: lhsT = transposed sorted head outputs
    (fp16 hi/lo), rhs = block-diagonal coefficient matrix A (fp16 hi/lo,
    built by mask-multiply on DVE/GPSIMD).
  * Two-supertile software pipeline: PE order is heads(0), heads(1),
    interp(0), interp(1); the DVE runs the 7-way sort of supertile st
    while the PE computes heads(st+1), so the PE never idles and the HAM
    clock gate stays warm.
  * Per-core output is written fp16 feature-major [HOR, B_core, QT]; the
    host converts/transposes to [B, HOR, QT] f32 when gathering.
"""

import dataclasses
from contextlib import ExitStack

import numpy as np

import concourse.bass as bass
import concourse.mybir as mybir
import concourse.tile as tile
from concourse import bass_utils
from concourse.bass import ts
from concourse.masks import make_identity

F32 = mybir.dt.float32
FP16 = mybir.dt.float16

B, T, D = 8192, 512, 512
H1, H2, HOR = 256, 128, 96
QF, QT = 7, 32
NCORES = 8
BC = B // NCORES  # batch per core
SUB = 512         # samples per super-tile
WSCALE = 64.0     # host pre-scale on weights (undone at activation evac)
QUANTILE_LEVELS = np.array(
    [0.025, 0.1, 0.25, 0.5, 0.75, 0.9, 0.975], dtype=np.float32
)

# optimal 16-CE sorting network for 7 elements (ascending), disjoint layers
SORT7_LAYERS = [
    [(1, 2), (3, 4), (5, 6)],
    [(0, 2), (3, 5), (4, 6)],
    [(0, 1), (4, 5), (2, 6)],
    [(0, 4), (1, 5)],
    [(0, 3), (2, 5)],
    [(1, 3), (2, 4)],
    [(2, 3)],
]


def batcher_layers(n):
    """Batcher odd-even mergesort compare-exchange layers (uniform distance)."""
    layers = []
    p = 1
    while p < n:
        k = p
        while k >= 1:
            pairs = []
            j = k % p
            while j + k < n:
                for i in range(min(k, n - j - k)):
                    lo, hi = i + j, i + j + k
                    if lo // (2 * p) == hi // (2 * p):
                        pairs.append((lo, hi))
                j += 2 * k
            if pairs:
                layers.append(pairs)
            k //= 2
        p *= 2
    return layers


def coalesce2(indices):
    """Coalesce sorted ints into 2-level lattices (start, ostep, ocount, icount)."""
    idx = list(indices)
    runs = []
    i = 0
    while i < len(idx):
        cnt = 1
        while i + cnt < len(idx) and idx[i + cnt] - idx[i + cnt - 1] == 1:
            cnt += 1
        runs.append((idx[i], cnt))
        i += cnt
    groups = []
    i = 0
    while i < len(runs):
        start, cnt = runs[i]
        j = i + 1
        if j < len(runs) and runs[j][1] == cnt:
            ostep = runs[j][0] - start
            k = j
            while (
                k < len(runs)
                and runs[k][1] == cnt
                and runs[k][0] - runs[k - 1][0] == ostep
            ):
                k += 1
            groups.append((start, ostep, k - i, cnt))
            i = k
        else:
            groups.append((start, 1, 1, cnt))
            i += 1
    return groups


def _view(ap, free_dims, extra_offset):
    """Rebuild an AP keeping its partition dim, with custom free-dim lattice."""
    dims = [tuple(ap.ap[0])] + [tuple(d) for d in free_dims]
    return dataclasses.replace(ap, ap=tuple(dims), offset=ap.offset + extra_offset)


def _host_constants(b1c, b2, b3):
    # bias_all [128, 32]: packed per-partition bias columns
    bias = np.zeros((128, 32), dtype=np.float32)
    for qh in range(QF):
        for mc in range(H1 // 128):
            bias[:, 2 * qh + mc] = b1c[qh, 128 * mc : 128 * (mc + 1)]
        bias[:, 14 + qh] = b2[qh]
        bias[:96, 21 + qh] = b3[qh]
    # M112 [112, 512]: block-diagonal 0/1 mask over (sample, coeff) x (sample, t)
    m112 = np.zeros((112, 512), dtype=np.float32)
    for s in range(16):
        m112[7 * s : 7 * s + 7, 32 * s : 32 * s + 32] = 1.0
    return bias, m112


def _split16(v):
    hi = v.astype(np.float16)
    lo = (v - hi.astype(np.float32)).astype(np.float16)
    return hi, lo


# ---------------------------------------------------------------------------
# device kernel
# ---------------------------------------------------------------------------

def _emit(ctx: ExitStack, tc: tile.TileContext, ins, outs, bc=BC):
    nc = tc.nc
    (xh_d, xl_d, q_d, w1h_d, w1l_d, w2h_d, w2l_d, w3h_d, w3l_d,
     bias_d, m112_d) = ins
    (r_d,) = outs
    n_sub = bc // SUB
    n_chunk = bc // 128
    ngrp = SUB // 16
    ql = QUANTILE_LEVELS

    cpool = ctx.enter_context(tc.tile_pool(name="cpool", bufs=1))
    wpool = ctx.enter_context(tc.tile_pool(name="wpool", bufs=1))
    qpool = ctx.enter_context(tc.tile_pool(name="qpool", bufs=1))
    xpool = ctx.enter_context(tc.tile_pool(name="xpool", bufs=1))
    h1pool = ctx.enter_context(tc.tile_pool(name="h1pool", bufs=1))
    h2pool = ctx.enter_context(tc.tile_pool(name="h2pool", bufs=1))
    fscr = ctx.enter_context(tc.tile_pool(name="fscr", bufs=3))
    o3pool = ctx.enter_context(tc.tile_pool(name="o3pool", bufs=9))
    sqgpool = ctx.enter_context(tc.tile_pool(name="sqgpool", bufs=2))
    sqTpool = ctx.enter_context(tc.tile_pool(name="sqTpool", bufs=4))
    apool = ctx.enter_context(tc.tile_pool(name="apool", bufs=4))
    rpool = ctx.enter_context(tc.tile_pool(name="rpool", bufs=4))
    tpsum = ctx.enter_context(tc.tile_pool(name="tpsum", bufs=2, space="PSUM"))
    hpsum = ctx.enter_context(tc.tile_pool(name="hpsum", bufs=3, space="PSUM"))
    rpsum = ctx.enter_context(tc.tile_pool(name="rpsum", bufs=3, space="PSUM"))

    # --- constants ---
    ident = cpool.tile([128, 128], F32)
    make_identity(nc, ident[:])
    bias_sb = cpool.tile([128, 32], F32)
    nc.sync.dma_start(bias_sb[:], bias_d)
    m112 = cpool.tile([112, 512], F32)
    nc.sync.dma_start(m112[:], m112_d)

    # PE warm-up
    warm_ps = tpsum.tile([128, 128], F32, tag="tps")
    nc.tensor.matmul(warm_ps[:], lhsT=ident[:], rhs=ident[:], start=True, stop=True)

    # --- input / weight DMAs, ordered so the PE can start early ---
    xh_sb = [[None] * (T // 128) for _ in range(n_sub)]
    xl_sb = [[None] * (T // 128) for _ in range(n_sub)]
    w1h_sb, w1l_sb = [], []
    for st in range(n_sub):
        for tci in range(T // 128):
            xh = xpool.tile([128, SUB], FP16, name=f"xh{st}_{tci}")
            nc.sync.dma_start(xh[:], xh_d[ts(tci, 128), ts(st, SUB)])
            xh_sb[st][tci] = xh
            xl = xpool.tile([128, SUB], FP16, name=f"xl{st}_{tci}")
            nc.sync.dma_start(xl[:], xl_d[ts(tci, 128), ts(st, SUB)])
            xl_sb[st][tci] = xl
        if st == 0:
            q_all = qpool.tile([128, n_chunk * QT], F32)
            nc.sync.dma_start(
                q_all[:].rearrange("p (c t) -> p c t", c=n_chunk),
                q_d.rearrange("(c p) t -> p c t", c=n_chunk),
            )
            for qh in range(QF):
                for (tag, lst, src) in (("h", w1h_sb, w1h_d),
                                        ("l", w1l_sb, w1l_d)):
                    w = wpool.tile([128, (D // 128) * H1], FP16,
                                   name=f"w1{tag}_{qh}")
                    nc.sync.dma_start(
                        w[:].rearrange("p (c m) -> p c m", c=D // 128),
                        src[qh].rearrange("(c p) m -> p c m", c=D // 128),
                    )
                    lst.append(w)

    w2h_sb, w2l_sb = [], []
    for qh in range(QF):
        for (tag, lst, src) in (("h", w2h_sb, w2h_d), ("l", w2l_sb, w2l_d)):
            w = wpool.tile([128, (H1 // 128) * H2], FP16, name=f"w2{tag}_{qh}")
            nc.sync.dma_start(
                w[:].rearrange("p (c m) -> p c m", c=H1 // 128),
                src[qh].rearrange("(c p) m -> p c m", c=H1 // 128),
            )
            lst.append(w)
    w3h_sb, w3l_sb = [], []
    for qh in range(QF):
        for (tag, lst, src) in (("h", w3h_sb, w3h_d), ("l", w3l_sb, w3l_d)):
            w = wpool.tile([128, HOR], FP16, name=f"w3{tag}_{qh}")
            nc.sync.dma_start(w[:], src[qh])
            lst.append(w)

    # =====================================================================
    # q path: per-row sort over QT on the DVE (overlaps the weight DMAs)
    # =====================================================================
    qscratch = qpool.tile([128, n_chunk * QT], F32)
    for layer in batcher_layers(QT):
        dist = layer[0][1] - layer[0][0]
        for (start, ostep, ocount, icount) in coalesce2(
            sorted(a for a, _ in layer)
        ):
            fd = [(QT, n_chunk), (ostep, ocount), (1, icount)]
            lo = _view(q_all[:], fd, start)
            hi = _view(q_all[:], fd, start + dist)
            sc = _view(qscratch[:], [(QT, n_chunk), (icount, ocount), (1, icount)], 0)
            nc.vector.tensor_tensor(sc, lo, hi, op=mybir.AluOpType.min)
            nc.vector.tensor_tensor(hi, lo, hi, op=mybir.AluOpType.max)
            nc.vector.tensor_copy(lo, sc)

    # =====================================================================
    # emission helpers
    # =====================================================================

    def emit_qT_ain(st):
        """Transpose sorted q (PE) and build ain [QT, 7*SUB] f32 on the DVE."""
        qT = qpool.tile([QT, SUB], F32, name=f"qT{st}", tag="qT", bufs=2)
        for c in range(SUB // 128):
            ps = tpsum.tile([QT, 128], F32, tag="tps")
            nc.tensor.matmul(
                ps[:], lhsT=q_all[:, ts(st * (SUB // 128) + c, QT)],
                rhs=ident[:], start=True, stop=True,
            )
            nc.scalar.copy(qT[:, ts(c, 128)], ps[:])

        ain = qpool.tile([QT, 7 * SUB], F32, name=f"ain{st}", tag="ain", bufs=2)

        def ain_view(i):
            return _view(ain[:], [(7, SUB)], i)

        f_prev = None
        for i in range(1, 7):
            lam = float(ql[i - 1])
            inv = float(
                np.float32(1.0)
                / (np.float32(ql[i] - ql[i - 1]) + np.float32(1e-8))
            )
            u = qpool.tile([QT, SUB], F32, name=f"u{st}_{i}", tag="utile", bufs=2)
            nc.vector.tensor_scalar(
                u[:], qT[:], lam, inv,
                mybir.AluOpType.subtract, mybir.AluOpType.mult,
            )
            f = qpool.tile([QT, SUB], F32, name=f"f{st}_{i}", tag=f"fp{i % 2}",
                           bufs=1)
            nc.vector.tensor_scalar(
                f[:], u[:], 1.0, 0.0, mybir.AluOpType.min, mybir.AluOpType.max
            )
            if i == 1:
                nc.vector.tensor_scalar(
                    ain_view(0), f[:], -1.0, 1.0,
                    mybir.AluOpType.mult, mybir.AluOpType.add,
                )
            else:
                nc.vector.tensor_tensor(
                    ain_view(i - 1), f_prev[:], f[:], op=mybir.AluOpType.subtract
                )
            f_prev = f
        nc.vector.tensor_copy(ain_view(6), f_prev[:])
        return ain

    def comp_mm(ps, whi, wlo, xhi, xlo, nk, first, last):
        """Emit the 3-pass compensated accumulation group over nk K-chunks.
        whi/wlo/xhi/xlo: callables chunk-index -> AP."""
        seq = (
            [("hh", c) for c in range(nk)]
            + [("hl", c) for c in range(nk)]
            + [("lh", c) for c in range(nk)]
        )
        for j, (kind, c) in enumerate(seq):
            lhs = whi(c) if kind[0] == "h" else wlo(c)
            rhs = xhi(c) if kind[1] == "h" else xlo(c)
            nc.tensor.matmul(
                ps, lhsT=lhs, rhs=rhs,
                start=(first and j == 0), stop=(last and j == len(seq) - 1),
            )

    def emit_heads(st):
        """h1 -> h2 -> o3 for one supertile, compensated fp16 on the PE."""
        h1h = [[None] * 2 for _ in range(QF)]
        h1l = [[None] * 2 for _ in range(QF)]
        for qh in range(QF):
            for mc in range(H1 // 128):
                ps = hpsum.tile([128, SUB], F32, tag="hps")
                comp_mm(
                    ps[:],
                    lambda c, qh=qh, mc=mc: w1h_sb[qh][:, ts(c * 2 + mc, 128)],
                    lambda c, qh=qh, mc=mc: w1l_sb[qh][:, ts(c * 2 + mc, 128)],
                    lambda c, st=st: xh_sb[st][c][:],
                    lambda c, st=st: xl_sb[st][c][:],
                    4, True, True,
                )
                bcol = bias_sb[:, 2 * qh + mc : 2 * qh + mc + 1]
                hh = h1pool.tile([128, SUB], FP16, name=f"h1h_{st}_{qh}_{mc}",
                                 tag=f"h1h_{qh}_{mc}", bufs=1)
                nc.scalar.activation(
                    hh[:], ps[:], mybir.ActivationFunctionType.Relu,
                    bias=bcol, scale=1.0 / WSCALE,
                )
                hf = fscr.tile([128, SUB], F32, tag="hfull")
                nc.scalar.activation(
                    hf[:], ps[:], mybir.ActivationFunctionType.Relu,
                    bias=bcol, scale=1.0 / WSCALE,
                )
                hl = h1pool.tile([128, SUB], FP16, name=f"h1l_{st}_{qh}_{mc}",
                                 tag=f"h1l_{qh}_{mc}", bufs=1)
                nc.vector.tensor_tensor(
                    hl[:], hf[:], hh[:], op=mybir.AluOpType.subtract
                )
                h1h[qh][mc] = hh
                h1l[qh][mc] = hl
        del hf, hl, hh

        h2h = [None] * QF
        h2l = [None] * QF
        for qh in range(QF):
            ps = hpsum.tile([128, SUB], F32, tag="hps")
            for mc in range(H1 // 128):
                comp_mm(
                    ps[:],
                    lambda c, qh=qh, mc=mc: w2h_sb[qh][:, ts(mc, H2)],
                    lambda c, qh=qh, mc=mc: w2l_sb[qh][:, ts(mc, H2)],
                    lambda c, qh=qh, mc=mc: h1h[qh][mc][:],
                    lambda c, qh=qh, mc=mc: h1l[qh][mc][:],
                    1, mc == 0, mc == 1,
                )
            bcol = bias_sb[:, 14 + qh : 15 + qh]
            hh = h2pool.tile([128, SUB], FP16, name=f"h2h_{st}_{qh}",
                             tag=f"h2h_{qh}", bufs=1)
            nc.scalar.activation(
                hh[:], ps[:], mybir.ActivationFunctionType.Relu,
                bias=bcol, scale=1.0 / WSCALE,
            )
            hf = fscr.tile([128, SUB], F32, tag="hfull")
            nc.scalar.activation(
                hf[:], ps[:], mybir.ActivationFunctionType.Relu,
                bias=bcol, scale=1.0 / WSCALE,
            )
            hl = h2pool.tile([128, SUB], FP16, name=f"h2l_{st}_{qh}",
                             tag=f"h2l_{qh}", bufs=1)
            nc.gpsimd.tensor_tensor(
                hl[:], hf[:], hh[:], op=mybir.AluOpType.subtract
            )
            h2h[qh] = hh
            h2l[qh] = hl

        o3 = [None] * QF
        for qh in range(QF):
            ps = hpsum.tile([HOR, SUB], F32, tag="hps")
            comp_mm(
                ps[:],
                lambda c, qh=qh: w3h_sb[qh][:, :],
                lambda c, qh=qh: w3l_sb[qh][:, :],
                lambda c, qh=qh: h2h[qh][:],
                lambda c, qh=qh: h2l[qh][:],
                1, True, True,
            )
            o = o3pool.tile([HOR, SUB], F32, name=f"o3_{st}_{qh}", tag="sortt")
            nc.scalar.activation(
                o[:], ps[:], mybir.ActivationFunctionType.Identity,
                bias=bias_sb[:HOR, 21 + qh : 22 + qh], scale=1.0 / WSCALE,
            )
            o3[qh] = o
        return o3

    def make_sort(st, o3):
        """7-element sort network on the DVE; final values land interleaved
        in SQG [96, 32*112] (free idx = 112*g + 7*s + i).  Returns
        (sqg_tile, generator) — each generator step emits one
        compare-exchange (2 DVE ops) so the caller can interleave them
        with other DVE work."""
        sqg = sqgpool.tile([HOR, ngrp * 112], F32, name=f"sqg{st}", tag="sqg")
        last_touch = {}
        for li, layer in enumerate(SORT7_LAYERS):
            for (a, b) in layer:
                last_touch[a] = (li, a, b)
                last_touch[b] = (li, a, b)
        cur = {k: o3[k] for k in range(QF)}

        def sqg_slot(j):
            return _view(sqg[:], [(112, ngrp), (7, 16)], j)

        def gen():
            ce_idx = 0
            for li, layer in enumerate(SORT7_LAYERS):
                for (a, b) in layer:
                    ia = cur[a][:].rearrange("p (g s) -> p g s", g=ngrp)
                    ib = cur[b][:].rearrange("p (g s) -> p g s", g=ngrp)
                    a_final = last_touch[a] == (li, a, b)
                    b_final = last_touch[b] == (li, a, b)
                    if a_final:
                        oa = sqg_slot(a)
                    else:
                        ta = o3pool.tile([HOR, SUB], F32, name=f"s{st}_{ce_idx}a",
                                         tag="sortt")
                        oa = ta[:].rearrange("p (g s) -> p g s", g=ngrp)
                    if b_final:
                        ob = sqg_slot(b)
                    else:
                        tb = o3pool.tile([HOR, SUB], F32, name=f"s{st}_{ce_idx}b",
                                         tag="sortt")
                        ob = tb[:].rearrange("p (g s) -> p g s", g=ngrp)
                    nc.vector.tensor_tensor(oa, ia, ib, op=mybir.AluOpType.min)
                    nc.vector.tensor_tensor(ob, ia, ib, op=mybir.AluOpType.max)
                    if not a_final:
                        cur[a] = ta
                    if not b_final:
                        cur[b] = tb
                    ce_idx += 1
                    yield

        return sqg, gen()

    def emit_interp(st, sqg, ain, bg_sort=None):
        """Per 16-sample group: PE transposes into one psum tile, scalar
        evacuation, A build alternating DVE/GPSIMD, one fp32 interp
        matmul, scalar r evacuation.  bg_sort: a sort generator whose
        compare-exchanges are interleaved one per group so the next
        supertile's sort shares the DVE fairly with the A builds."""
        for g in range(ngrp):
            gg = st * ngrp + g  # global group index
            if bg_sort is not None:
                next(bg_sort, None)

            ps_t = tpsum.tile([112, 128], F32, tag="tps")
            nc.tensor.matmul(
                ps_t[:, :HOR], lhsT=sqg[:, 112 * g : 112 * (g + 1)],
                rhs=ident[:HOR, :HOR], start=True, stop=True,
            )
            nc.tensor.matmul(
                ps_t[:, HOR:128], lhsT=_view(ain[:], [(1, 112)], 112 * g),
                rhs=ident[:QT, :QT], start=True, stop=True,
            )
            sqa = sqTpool.tile([112, 128], F32, tag="sqa")
            nc.scalar.copy(sqa[:], ps_t[:])

            # A [112, 512] = broadcast(aT) * M112, alternating DVE/GPSIMD
            A = apool.tile([112, 512], F32, tag="A")
            av = sqa[:, HOR:128].unsqueeze(1).broadcast_to((112, 16, QT))
            mv = m112[:].rearrange("p (s t) -> p s t", s=16)
            Av = A[:].rearrange("p (s t) -> p s t", s=16)
            eng = nc.vector if g % 2 == 0 else nc.gpsimd
            eng.tensor_tensor(Av, av, mv, op=mybir.AluOpType.mult)

            rps = rpsum.tile([HOR, 512], F32, tag="rps")
            nc.tensor.matmul(
                rps[:], lhsT=sqa[:, :HOR], rhs=A[:], start=True, stop=True
            )

            r_sb = rpool.tile([HOR, 512], FP16, tag="rsb")
            nc.scalar.copy(r_sb[:], rps[:])
            nc.sync.dma_start(
                r_d[:, 16 * gg : 16 * (gg + 1), :],
                r_sb[:].rearrange("p (s t) -> p s t", s=16),
            )

    # =====================================================================
    # pipelined emission
    # =====================================================================
    ain = [None] * n_sub
    o3_0 = emit_heads(0)
    for st in range(n_sub):
        ain[st] = emit_qT_ain(st)
    sqg_0, gen_0 = make_sort(0, o3_0)
    for _ in gen_0:
        pass
    o3_1 = emit_heads(1)
    sqg_1, gen_1 = make_sort(1, o3_1)
    emit_interp(0, sqg_0, ain[0], bg_sort=gen_1)
    emit_interp(1, sqg_1, ain[1])


# Per-instruction-type sync-wait slot capacity in the walrus ISA descriptors.
_WAIT_CAPACITY = {}  # default: every type gets a single wait slot
_DRAIN_CAPACITY = {
    "EngineType.SP": 1,
    "EngineType.PE": 1,
}


def _split_waits(nc):
    """Some walrus ISA descriptors (LDWEIGHTS, DMA) have too few sync-wait
    slots for the waits Tile emits.  Move surplus waits of overflowing
    instructions onto drains inserted right before them on the same queue."""
    for fn in nc.m.functions:
        for blk in fn.blocks:
            insts = list(blk.instructions)
            out = []
            changed = False
            for ins in insts:
                si = ins.sync_info
                cap = _WAIT_CAPACITY.get(type(ins).__name__, 1)
                if si is not None and si.on_wait and len(si.on_wait) > cap:
                    waits = list(si.on_wait)
                    surplus = waits[:-cap]
                    dcap = _DRAIN_CAPACITY.get(str(ins.engine), 1)
                    di = 0
                    while surplus:
                        chunk, surplus = surplus[:dcap], surplus[dcap:]
                        out.append(
                            mybir.InstDrain(
                                name=f"{ins.name}-wfence{di}",
                                engine=ins.engine,
                                ins=[],
                                outs=[],
                                sync_info=mybir.SyncInfo(
                                    on_wait=chunk, on_update=[]
                                ),
                            )
                        )
                        di += 1
                    si.on_wait = waits[-cap:]
                    changed = True
                out.append(ins)
            if changed:
                blk.instructions = out


def build_module(bc=BC):
    nc = bass.Bass("TRN2", target_bir_lowering=False, debug=False)
    xh_d = nc.dram_tensor("xT_hi", [T, bc], FP16, kind="ExternalInput").ap()
    xl_d = nc.dram_tensor("xT_lo", [T, bc], FP16, kind="ExternalInput").ap()
    q_d = nc.dram_tensor("q", [bc, QT], F32, kind="ExternalInput").ap()
    w1h_d = nc.dram_tensor("W1hi", [QF, D, H1], FP16, kind="ExternalInput").ap()
    w1l_d = nc.dram_tensor("W1lo", [QF, D, H1], FP16, kind="ExternalInput").ap()
    w2h_d = nc.dram_tensor("W2hi", [QF, H1, H2], FP16, kind="ExternalInput").ap()
    w2l_d = nc.dram_tensor("W2lo", [QF, H1, H2], FP16, kind="ExternalInput").ap()
    w3h_d = nc.dram_tensor("W3hi", [QF, H2, HOR], FP16, kind="ExternalInput").ap()
    w3l_d = nc.dram_tensor("W3lo", [QF, H2, HOR], FP16, kind="ExternalInput").ap()
    bias_d = nc.dram_tensor("bias_all", [128, 32], F32, kind="ExternalInput").ap()
    m112_d = nc.dram_tensor("m112", [112, 512], F32, kind="ExternalInput").ap()
    r_d = nc.dram_tensor("r_out", [HOR, bc, QT], FP16, kind="ExternalOutput").ap()

    with tile.TileContext(nc) as tc:
        with ExitStack() as ctx:
            _emit(ctx, tc,
                  (xh_d, xl_d, q_d, w1h_d, w1l_d, w2h_d, w2l_d, w3h_d, w3l_d,
                   bias_d, m112_d),
                  (r_d,), bc=bc)
    _split_waits(nc)
    return nc


_NC_CACHE = {}
LAST_EXEC_TIME_NS = None


def kernel(**inputs) -> np.ndarray:
    global LAST_EXEC_TIME_NS
    x = np.asarray(inputs["x"], dtype=np.float32)
    q = np.asarray(inputs["q"], dtype=np.float32)
    w_bb = np.asarray(inputs["W_bb"], dtype=np.float64)
    b_bb = np.asarray(inputs["b_bb"], dtype=np.float64)
    w1 = np.asarray(inputs["W1"], dtype=np.float64)
    b1 = np.asarray(inputs["b1"], dtype=np.float64)
    w2 = np.asarray(inputs["W2"], dtype=np.float32)
    w3 = np.asarray(inputs["W3"], dtype=np.float32)

    # Fold the backbone into the first head layer (float64 on the host).
    w1c = (w_bb[None, :, :] @ w1).astype(np.float32)
    b1c = np.ascontiguousarray((b_bb @ w1 + b1).astype(np.float32))

    w1hi, w1lo = _split16(w1c * WSCALE)
    w2hi, w2lo = _split16(w2 * WSCALE)
    w3hi, w3lo = _split16(w3 * WSCALE)

    bias, m112 = _host_constants(
        b1c,
        np.asarray(inputs["b2"], dtype=np.float32),
        np.asarray(inputs["b3"], dtype=np.float32),
    )

    if BC not in _NC_CACHE:
        _NC_CACHE[BC] = build_module(BC)
    nc = _NC_CACHE[BC]

    in_maps = []
    for c in range(NCORES):
        xT = np.ascontiguousarray(x[BC * c : BC * (c + 1)].T)
        xhi, xlo = _split16(xT)
        in_maps.append(
            {
                "xT_hi": xhi,
                "xT_lo": xlo,
                "q": np.ascontiguousarray(q[BC * c : BC * (c + 1)]),
                "W1hi": w1hi, "W1lo": w1lo,
                "W2hi": w2hi, "W2lo": w2lo,
                "W3hi": w3hi, "W3lo": w3lo,
                "bias_all": bias,
                "m112": m112,
            }
        )

    res = bass_utils.run_bass_kernel_spmd(nc, in_maps, core_ids=list(range(NCORES)))
    LAST_EXEC_TIME_NS = res.exec_time_ns
    out = np.empty((B, HOR, QT), dtype=np.float32)
    for c in range(NCORES):
        out[BC * c : BC * (c + 1)] = np.transpose(
            res.results[c]["r_out"].astype(np.float32), (1, 0, 2)
        )
    return out
